# revision 1
# baseline (speedup 1.0000x reference)
"""Batched 20x20 SPD covariance-matrix inversion on 8 Trainium2 NeuronCores.

For each of 131072 batches: build C = exp(-1.5 * pairwise_dist(pos)) + 0.01*I
from 20 2-D points, return C^{-1}.

Strategy (per core, data-parallel over batch):
 - batch-major layout: each of 128 SBUF partitions holds Mg matrices' full
   20x20 (400 fp32) in the free dim; N_STREAMS independent streams.
 - symmetric sweep operator (Gauss-Jordan preserving symmetry): only the
   upper triangle is updated each pivot, covered by 4 row-band rectangles.
 - gather-free sweep: the raw pivot column/row is read straight out of A
   as broadcast operands of the rank-1 muls (all muls are emitted before
   any in-place sub so the subs never race those reads), cr = c/pivot is
   computed from A the same way, and the reciprocal of the NEXT pivot is
   issued right after the sub that finalizes its diagonal element. The
   pivot row itself is excluded from the update (its band splits around
   it): those results are discarded by the pivot row/col rewrite anyway.
 - engine split: the rank-1 updates (2 tensor-tensor passes per element)
   are split row-wise between DVE and GPSIMD plain tensor_tensor ops
   (ScalarTensorTensor is BIR-capped at 3D, so unusable here); ACT does
   the cov-build square/sqrt/exp, the pivot row/col writes, and the
   mirror of the upper triangle into the lower.
 - the final negation (sweep yields -A^{-1}) is folded into the last
   pivot's subtracts (reversed operands), so no extra negate pass runs.
 - ACTIVE streams are software-pipelined round-robin at pivot granularity:
   one stream's serial pivot prefix hides under the other's rank-1 work;
   cov builds and finalizes of adjacent streams overlap sweeps the same
   way. All pos DMAs are prefetched at kernel start; outputs are stored
   in m-halves so the first half's DMA overlaps the second's mirror.
"""

import numpy as np

import concourse.bass as bass  # noqa: F401  (registers engine APIs)
import concourse.tile as tile
from concourse import bacc, mybir
from concourse.bass_utils import run_bass_kernel_spmd

N = 20                  # matrix dim
D = 2                   # coord dim
PHI = 1.5
TAU = 0.01
P = 128                 # SBUF partitions
N_CORES = 8
B_TOTAL = 131072
B_CORE = B_TOTAL // N_CORES   # 16384

F32 = mybir.dt.float32
AF = mybir.ActivationFunctionType
OP = mybir.AluOpType

# Upper-triangle rectangle cover: rows [r0,r1) x cols [r0,N)
RECTS = [(0, 5), (5, 10), (10, 15), (15, 20)]

# --- engine-assignment knobs (autotuned via CoreSim) -----------------------
# per rect: how many of its rows (from the top) go to GPSIMD for the rank-1
# update; the rest go to DVE.
GP_ROWS = [5, 1, 1, 5]
# on even pivots one extra row per listed rect goes to GPSIMD (half-row
# granularity for the DVE/GPSIMD balance)
GP_ROWS_EVEN = [5, 1, 1, 5]
# per rect: cov-build tensor-tensor ops (dx, dy, add) engine: "v" DVE, "g" GP
COV_ENG = ["v", "g", "g", "v"]
# per rect: cov rows (from the top) built on GPSIMD; rest on DVE.
# [0,5,5,0] reproduces COV_ENG ["v","g","g","v"].
COV_GP_ROWS = [0, 5, 5, 2]
CR_ENG = "s2"            # cr = c * r:  "v" DVE tensor_mul, "g" GP stt
MIRROR_ENG = "a"        # "a" ACT copy(scale=-1) pre-negate, "v" DVE post
PIVOT_COPY_ENG = "a"    # pivot row/col <- cr copies: "v" DVE, "a" ACT, "g" GP
N_STREAMS = 8           # independent m-slices (Mg = B_CORE/P/N_STREAMS)
STREAM_SIZES = None     # optional per-stream m sizes (must sum to B_CORE/P)
ACTIVE = 2              # streams pipelined concurrently
STAGGER = 7             # yields to prime stream 0 before starting stream 1
COV_MERGED = False       # cov: one 2-coord sub (needs 2E dd tile) vs dx/dy
DIAG_ENG = "a"          # per-pivot diag<- -r + TAU add: "a" ACT, "v" DVE, "g" GP
COV_PAR_START = False    # run both initial streams' cov builds in parallel
COV_ENG0 = None         # optional cov engine mix for stream 0 (startup ramp)
TAU_ENG = "g"           # per-rect diag nugget add: "v" DVE, "g" GP


def _gp_mul(nc, out, a, b):
    """out = a * b on GPSIMD (plain TensorTensor: >=4D APs are BIR-legal,
    unlike ScalarTensorTensor which the BIR verifier caps at 3D)."""
    nc.gpsimd.tensor_tensor(out, a, b, OP.mult)


def _gp_sub(nc, out, a, b):
    """out = a - b on GPSIMD."""
    nc.gpsimd.tensor_tensor(out, a, b, OP.subtract)


def _gp_add(nc, out, a, b):
    nc.gpsimd.tensor_tensor(out, a, b, OP.add)


def emit_kernel(tc, pos_ap, out_ap, b_core, n_streams):
    """Emit the per-core program. pos: [b_core, 40] f32, out: [b_core, 400] f32."""
    nc = tc.nc
    m_total = b_core // P
    if STREAM_SIZES is not None:
        sizes = list(STREAM_SIZES)
        n_streams = len(sizes)
    else:
        sizes = [m_total // n_streams] * n_streams
    assert sum(sizes) == m_total
    offs = [0]
    for sz in sizes:
        offs.append(offs[-1] + sz)

    pos_r = pos_ap.rearrange("(p m) f -> p m f", p=P)
    out_r = out_ap.rearrange("(p m) f -> p m f", p=P)
    big_bufs = 2 if (ACTIVE <= 2 and max(sizes) < 32) else 1

    with (
        tc.tile_pool(name="pos", bufs=1) as pos_pool,
        tc.tile_pool(name="A", bufs=big_bufs) as a_pool,
        tc.tile_pool(name="cov", bufs=big_bufs) as cov_pool,
        tc.tile_pool(name="rect", bufs=1) as rect_pool,
        tc.tile_pool(name="grect", bufs=1) as grect_pool,
        tc.tile_pool(name="small", bufs=2) as small_pool,
    ):
        # prefetch every stream's positions up front
        pos_tiles = []
        for s in range(n_streams):
            pos_t = pos_pool.tile([P, sizes[s] * N * D], F32, tag=f"pos{s % 4}")
            nc.sync.dma_start(pos_t[:, :], pos_r[:, offs[s] : offs[s + 1], :])
            pos_tiles.append(pos_t)

        def stream_gen(s):
            par = s % ACTIVE
            Mg = sizes[s]
            posv = pos_tiles[s][:, :].rearrange(
                "p (m i d) -> p m i d", m=Mg, i=N
            )
            A = a_pool.tile([P, Mg * N * N], F32, tag=f"A{par}")
            A4 = A[:, :].rearrange("p (m i j) -> p m i j", m=Mg, i=N)
            Av = A[:, :].rearrange("p (m x) -> p m x", m=Mg)

            # ---- covariance build over the upper rect cover ----
            cov_eng = COV_ENG0 if (s == 0 and COV_ENG0) else COV_ENG
            for ri, (r0, r1) in enumerate(RECTS):
                nr, ncl = r1 - r0, N - r0
                reg = A4[:, :, r0:r1, r0:]
                sub_a = (
                    (lambda o, a, b: _gp_sub(nc, o, a, b))
                    if cov_eng[ri] == "g"
                    else nc.vector.tensor_sub
                )  # "m": subs on DVE, add on GP
                add_a = (
                    (lambda o, a, b: _gp_add(nc, o, a, b))
                    if cov_eng[ri] in ("g", "m")
                    else nc.vector.tensor_add
                )
                if COV_MERGED:
                    # dd[m,i,j,:] = p[i,:] - p[j,:] in ONE sub (both
                    # coords), square on ACT, strided add -> d^2 in A.
                    pi = (
                        posv[:, :, r0:r1, :]
                        .unsqueeze(3)
                        .broadcast_to([P, Mg, nr, ncl, D])
                    )
                    pj = (
                        posv[:, :, r0:, :]
                        .unsqueeze(2)
                        .broadcast_to([P, Mg, nr, ncl, D])
                    )
                    dd = cov_pool.tile(
                        [P, Mg * nr * ncl * D], F32, tag=f"dd{par}"
                    )
                    ddv = dd[:, :].rearrange(
                        "p (m i j d) -> p m i j d", m=Mg, i=nr, j=ncl
                    )
                    sub_a(ddv, pi, pj)
                    nc.scalar.square(dd[:, :], dd[:, :])
                    add_a(reg, ddv[:, :, :, :, 0], ddv[:, :, :, :, 1])
                else:
                    # dx into A (in-place square), dy in an E-sized tmp;
                    # rows split between GPSIMD (top COV_GP_ROWS[ri]) and
                    # DVE for fine-grained engine balance
                    dy = cov_pool.tile(
                        [P, Mg * nr * ncl], F32, tag=f"dd{par}"
                    )
                    dyv = dy[:, :].rearrange(
                        "p (m i j) -> p m i j", m=Mg, i=nr
                    )
                    csplit = r0 + COV_GP_ROWS[ri]
                    for ceng, pa, pb in (("g", r0, csplit), ("v", csplit, r1)):
                        pn = pb - pa
                        if pn <= 0:
                            continue
                        regp = A4[:, :, pa:pb, r0:]
                        dyp = dyv[:, :, pa - r0 : pb - r0]
                        xi = (
                            posv[:, :, pa:pb, 0]
                            .unsqueeze(3)
                            .broadcast_to([P, Mg, pn, ncl])
                        )
                        xj = (
                            posv[:, :, r0:, 0]
                            .unsqueeze(2)
                            .broadcast_to([P, Mg, pn, ncl])
                        )
                        yi = (
                            posv[:, :, pa:pb, 1]
                            .unsqueeze(3)
                            .broadcast_to([P, Mg, pn, ncl])
                        )
                        yj = (
                            posv[:, :, r0:, 1]
                            .unsqueeze(2)
                            .broadcast_to([P, Mg, pn, ncl])
                        )
                        if ceng == "g":
                            _gp_sub(nc, regp, xi, xj)
                            nc.scalar.square(regp, regp)
                            _gp_sub(nc, dyp, yi, yj)
                            nc.scalar.square(dyp, dyp)
                            _gp_add(nc, regp, regp, dyp)
                        else:
                            nc.vector.tensor_sub(regp, xi, xj)
                            nc.scalar.square(regp, regp)
                            nc.vector.tensor_sub(dyp, yi, yj)
                            nc.scalar.square(dyp, dyp)
                            nc.vector.tensor_add(regp, regp, dyp)
                nc.scalar.sqrt(reg, reg)
                nc.scalar.activation(reg, reg, AF.Exp, scale=-PHI)
                # nugget on this rect's diagonal segment only: pivot k's
                # reciprocal then depends on rect band(k) alone, letting
                # early pivots overlap the remaining rects' cov chains
                dseg = Av[:, :, r0 * (N + 1) : (r1 - 1) * (N + 1) + 1 : N + 1]
                if TAU_ENG == "g":
                    nc.gpsimd.tensor_scalar_add(dseg, dseg, TAU)
                else:
                    nc.vector.tensor_scalar_add(dseg, dseg, TAU)
                yield


            # ---- sweep all 20 pivots (gather-free) ----
            # The raw pivot column/row is read straight out of A as
            # broadcast operands of the rank-1 muls (column k for rows
            # above the pivot, row k for rows below); cr (= c * 1/pivot)
            # is computed from A the same way. All muls are emitted
            # before any sub so the in-place subs (which corrupt the
            # pivot row/col: cr[k] = 1) never race the raw reads.
            # The reciprocal for pivot k+1 is issued right after the sub
            # that finalizes A[k+1,k+1], hiding it under pivot k's tail.
            rK = small_pool.tile([P, Mg], F32, tag=f"r{par}")
            nc.vector.reciprocal(rK[:, :], A4[:, :, 0, 0])
            for k in range(N):
                crK = small_pool.tile([P, Mg * N], F32, tag=f"cr{par}")
                cr3 = crK[:, :].rearrange("p (m i) -> p m i", m=Mg)
                rb = rK[:, :].unsqueeze(2).broadcast_to([P, Mg, N])

                def crmul(which, o, a, b):
                    # "s": col part on GPSIMD; "s2": col part alternates
                    # engines by pivot parity (finer-grained balance)
                    if which == 0 and (
                        CR_ENG == "s" or (CR_ENG == "s2" and k % 2 == 0)
                    ):
                        _gp_mul(nc, o, a, b)
                    elif CR_ENG == "g":
                        _gp_mul(nc, o, a, b)
                    else:
                        nc.vector.tensor_mul(o, a, b)

                if k:
                    crmul(0, cr3[:, :, :k], A4[:, :, :k, k], rb[:, :, :k])
                crmul(1, cr3[:, :, k:], A4[:, :, k, k:], rb[:, :, k:])

                def c_raw(a, b):
                    """Broadcast AP of raw c[a:b] read from A's storage."""
                    if b <= k + 1:  # rows at or above the pivot: column k
                        return A4[:, :, a:b, k].unsqueeze(3)
                    return A4[:, :, k, a:b].unsqueeze(3)  # below: row k

                last = k == N - 1
                subs = []
                gp_rows = GP_ROWS_EVEN if k % 2 == 0 else GP_ROWS
                for ri, (r0, r1) in enumerate(RECTS):
                    ncl = N - r0
                    gsplit = r0 + gp_rows[ri]
                    for eng, a, b in (("g", r0, gsplit), ("v", gsplit, r1)):
                        if b <= a:
                            continue
                        # rows to update: [a,b) minus the pivot row k
                        # (its results are discarded: the pivot row/col
                        # copies rewrite it below). Rows above the pivot
                        # read c from column k, rows below from row k.
                        if a <= k < b:
                            pieces = [(a, k), (k + 1, b)]
                        else:
                            pieces = [(a, b)]
                        pieces = [(pa, pb) for pa, pb in pieces if pb > pa]
                        if not pieces:
                            continue
                        tot = sum(pb - pa for pa, pb in pieces)
                        pool = grect_pool if eng == "g" else rect_pool
                        tmp = pool.tile(
                            [P, Mg * tot * ncl], F32, tag=f"t{eng}{par}r{ri}"
                        )
                        tv = tmp[:, :].rearrange(
                            "p (m i j) -> p m i j", m=Mg, i=tot
                        )
                        mul = (
                            (lambda o, x, y: _gp_mul(nc, o, x, y))
                            if eng == "g"
                            else nc.vector.tensor_mul
                        )
                        ofs = 0
                        for (pa, pb) in pieces:
                            pn = pb - pa
                            crb = (
                                cr3[:, :, r0:]
                                .unsqueeze(2)
                                .broadcast_to([P, Mg, pn, ncl])
                            )
                            mul(
                                tv[:, :, ofs : ofs + pn],
                                c_raw(pa, pb).broadcast_to([P, Mg, pn, ncl]),
                                crb,
                            )
                            # does this finalize next pivot's diagonal?
                            owns_next = pa <= k + 1 < pb
                            subs.append(
                                (
                                    eng,
                                    A4[:, :, pa:pb, r0:],
                                    tv[:, :, ofs : ofs + pn],
                                    owns_next,
                                )
                            )
                            ofs += pn
                subs.sort(key=lambda t: not t[3])  # next-diag owner first
                for si, (eng, reg, tv, owns_next) in enumerate(subs):
                    if eng == "g":
                        if last:
                            # fold the final negation into the last
                            # pivot: reg <- tv - reg = -(reg - tv)
                            _gp_sub(nc, reg, tv, reg)
                        else:
                            _gp_sub(nc, reg, reg, tv)
                    else:
                        if last:
                            nc.vector.tensor_sub(reg, tv, reg)
                        else:
                            nc.vector.tensor_sub(reg, reg, tv)
                    if owns_next and not last:
                        rK_next = small_pool.tile(
                            [P, Mg], F32, tag=f"r{par}"
                        )
                        nc.vector.reciprocal(
                            rK_next[:, :], A4[:, :, k + 1, k + 1]
                        )
                # pivot row/col (upper parts) <- cr; diag <- -r
                # (the last pivot writes negated values: the whole rect
                # cover holds -result after its reverse subtract)
                if k < N - 1:
                    pcopy = {
                        "v": nc.vector.tensor_copy,
                        "a": nc.scalar.copy,
                        "g": nc.gpsimd.tensor_copy,
                    }[PIVOT_COPY_ENG]
                    if k:
                        pcopy(A4[:, :, :k, k], cr3[:, :, :k])
                    pcopy(A4[:, :, k, k + 1 :], cr3[:, :, k + 1 :])
                    if DIAG_ENG == "a":
                        nc.scalar.mul(A4[:, :, k, k], rK[:, :], -1.0)
                    elif DIAG_ENG == "g":
                        nc.gpsimd.tensor_scalar_mul(
                            A4[:, :, k, k], rK[:, :], -1.0
                        )
                    else:
                        nc.vector.tensor_scalar_mul(
                            A4[:, :, k, k], rK[:, :], -1.0
                        )
                else:
                    if PIVOT_COPY_ENG == "a":
                        nc.scalar.mul(A4[:, :, :k, k], cr3[:, :, :k], -1.0)
                    else:
                        nc.vector.tensor_scalar_mul(
                            A4[:, :, :k, k], cr3[:, :, :k], -1.0
                        )
                    nc.vector.tensor_copy(A4[:, :, k, k], rK[:, :])
                if k < N - 1:
                    rK = rK_next
                yield

            # ---- finalize: mirror upper -> lower (values already negated),
            # in m-halves so the first half's store overlaps the second
            # half's mirror. The last stream has no concurrent work left,
            # so its mirror runs split across the otherwise-idle DVE+GPSIMD.
            tail = s == n_streams - 1
            h = Mg // 2
            for hi, (m0, m1) in enumerate(((0, h), (h, Mg))):
                for i in range(N - 1):
                    if tail:
                        mcopy = (
                            nc.vector.tensor_copy
                            if i % 2
                            else nc.gpsimd.tensor_copy
                        )
                    elif MIRROR_ENG == "a":
                        mcopy = nc.scalar.copy
                    else:
                        mcopy = nc.vector.tensor_copy
                    mcopy(
                        A4[:, m0:m1, i + 1 :, i], A4[:, m0:m1, i, i + 1 :]
                    )
                # the last stream's second half goes out via the ACT
                # HWDGE queue so both halves' transfers overlap (ACT is
                # idle during the pipeline drain; mid-run streams stay on
                # the SP queue where the issue cost is off-engine)
                dma_eng = nc.scalar if (tail and hi == 1) else nc.sync
                dma_eng.dma_start(
                    out_r[:, offs[s] + m0 : offs[s] + m1, :],
                    A[:, m0 * N * N : m1 * N * N],
                )
                yield

        pending = list(range(n_streams))
        active = [stream_gen(pending.pop(0))]
        if COV_PAR_START and pending:
            # run both initial streams' cov builds in parallel, then
            # prime stream 0's sweep so pivots stay phase-offset
            active.append(stream_gen(pending.pop(0)))
            for _ in range(len(RECTS)):
                for gen in active:
                    next(gen)
            for _ in range(STAGGER):
                next(active[0])
        else:
            # prime the first stream so concurrent streams stay offset
            for _ in range(STAGGER):
                next(active[0])
        while pending or active:
            while len(active) < ACTIVE and pending:
                active.append(stream_gen(pending.pop(0)))
            for gen in list(active):
                try:
                    next(gen)
                except StopIteration:
                    active.remove(gen)


_CACHE = {}


def build_nc(b_core=B_CORE, n_streams=None, num_devices=N_CORES):
    if n_streams is None:
        n_streams = N_STREAMS
    key = (b_core, n_streams, num_devices)
    if key in _CACHE:
        return _CACHE[key]
    nc = bacc.Bacc(
        "TRN2", target_bir_lowering=False, debug=False, num_devices=num_devices
    )
    pos_d = nc.dram_tensor("pos", [b_core, N * D], F32, kind="ExternalInput")
    out_d = nc.dram_tensor("out", [b_core, N * N], F32, kind="ExternalOutput")
    with tile.TileContext(nc) as tc:
        emit_kernel(tc, pos_d.ap(), out_d.ap(), b_core, n_streams)
    nc.compile()
    _CACHE[key] = nc
    return nc


def run(pos_full, b_core=B_CORE, n_streams=None, n_cores=N_CORES, **kw):
    """pos_full: [n_cores*b_core, 20, 2] f32 -> [n_cores*b_core, 20, 20] f32."""
    nc = build_nc(b_core, n_streams, n_cores)
    flat = np.ascontiguousarray(
        np.asarray(pos_full, dtype=np.float32).reshape(-1, N * D)
    )
    in_maps = [
        {"pos": flat[i * b_core : (i + 1) * b_core]} for i in range(n_cores)
    ]
    res = run_bass_kernel_spmd(nc, in_maps, core_ids=list(range(n_cores)), **kw)
    out = np.concatenate([r["out"] for r in res.results], axis=0)
    return out.reshape(-1, N, N), res


def kernel(neighbor_positions, edge_list=None):
    out, _ = run(neighbor_positions)
    return out



# revision 19
# speedup vs baseline: 1.1114x; 1.1114x over previous
"""Batched 20x20 SPD covariance-matrix inversion on 8 Trainium2 NeuronCores.

For each of 131072 batches: build C = exp(-1.5 * pairwise_dist(pos)) + 0.01*I
from 20 2-D points, return C^{-1}.

Strategy (per core, data-parallel over batch):
 - batch-major layout: each of 128 SBUF partitions holds Mg matrices' full
   20x20 (400 fp32) in the free dim; N_STREAMS independent streams.
 - symmetric sweep operator (Gauss-Jordan preserving symmetry): only the
   upper triangle is updated each pivot, covered by 4 row-band rectangles.
 - gather-free sweep: the raw pivot column/row is read straight out of A
   as broadcast operands of the rank-1 muls (all muls are emitted before
   any in-place sub so the subs never race those reads), cr = c/pivot is
   computed from A the same way, and the reciprocal of the NEXT pivot is
   issued right after the sub that finalizes its diagonal element. The
   pivot row itself is excluded from the update (its band splits around
   it): those results are discarded by the pivot row/col rewrite anyway.
 - engine split: the rank-1 updates (2 tensor-tensor passes per element)
   are split row-wise between DVE and GPSIMD plain tensor_tensor ops
   (ScalarTensorTensor is BIR-capped at 3D, so unusable here); ACT does
   the cov-build square/sqrt/exp, the pivot row/col writes, and the
   mirror of the upper triangle into the lower.
 - the final negation (sweep yields -A^{-1}) is folded into the last
   pivot's subtracts (reversed operands), so no extra negate pass runs.
 - ACTIVE streams are software-pipelined round-robin at pivot granularity:
   one stream's serial pivot prefix hides under the other's rank-1 work;
   cov builds and finalizes of adjacent streams overlap sweeps the same
   way. All pos DMAs are prefetched at kernel start; outputs are stored
   in m-halves so the first half's DMA overlaps the second's mirror.
"""

import numpy as np

import concourse.bass as bass  # noqa: F401  (registers engine APIs)
import concourse.tile as tile
from concourse import bacc, mybir
from concourse.bass_utils import run_bass_kernel_spmd

N = 20                  # matrix dim
D = 2                   # coord dim
PHI = 1.5
TAU = 0.01
P = 128                 # SBUF partitions
N_CORES = 8
B_TOTAL = 131072
B_CORE = B_TOTAL // N_CORES   # 16384

F32 = mybir.dt.float32
AF = mybir.ActivationFunctionType
OP = mybir.AluOpType

# Upper-triangle rectangle cover: rows [r0,r1) x cols [r0,N)
RECTS = [(0, 5), (5, 10), (10, 15), (15, 20)]

# --- engine-assignment knobs (autotuned via CoreSim) -----------------------
# per rect: how many of its rows (from the top) go to GPSIMD for the rank-1
# update; the rest go to DVE.
GP_ROWS = [5, 1, 1, 5]
# on even pivots one extra row per listed rect goes to GPSIMD (half-row
# granularity for the DVE/GPSIMD balance)
GP_ROWS_EVEN = [5, 1, 1, 5]
# per rect: cov-build tensor-tensor ops (dx, dy, add) engine: "v" DVE, "g" GP
COV_ENG = ["v", "g", "g", "v"]
# per rect: cov rows (from the top) built on GPSIMD; rest on DVE.
# [0,5,5,0] reproduces COV_ENG ["v","g","g","v"].
COV_GP_ROWS = [0, 5, 5, 2]
CR_ENG = "s2"            # cr = c * r:  "v" DVE tensor_mul, "g" GP stt
MIRROR_ENG = "a"        # "a" ACT copy(scale=-1) pre-negate, "v" DVE post
PIVOT_COPY_ENG = "a"    # pivot row/col <- cr copies: "v" DVE, "a" ACT, "g" GP
N_STREAMS = 8           # independent m-slices (Mg = B_CORE/P/N_STREAMS)
STREAM_SIZES = None     # optional per-stream m sizes (must sum to B_CORE/P)
ACTIVE = 2              # streams pipelined concurrently
STAGGER = 6             # yields to prime stream 0 before starting stream 1
COV_MERGED = False       # cov: one 2-coord sub (needs 2E dd tile) vs dx/dy
DIAG_ENG = "a"          # per-pivot diag<- -r + TAU add: "a" ACT, "v" DVE, "g" GP
COV_PAR_START = False    # run both initial streams' cov builds in parallel
COV_ENG0 = None         # optional cov engine mix for stream 0 (startup ramp)
TAU_ENG = "g"           # per-rect diag nugget add: "v" DVE, "g" GP
# batch all rects' d^2 first, then all sqrts, then all exps: the ACT
# function table holds square+copy in every set but sqrt and exp live in
# different sets, so interleaving sqrt/exp per rect forces a 1283ns
# LoadActFuncSet per switch (~57 loads); phased order pays 2 per stream.
COV_PHASED = False
# Fine-grained rank-1 cover: list of (r0, r1, eng) bands, each updating
# rows [r0,r1) x cols [r0,N).  Narrow bands with their own column start
# carry far less sub-diagonal garbage than the 4x5 rect cover (whose
# lower rows update cols from the parent rect's r0).  GPSIMD tensor ops
# have no per-instruction engine overhead in the cost model, so it takes
# the narrow bottom bands; DVE (60ns init per instr) keeps wide top ones.
# None falls back to the baseline RECTS/GP_ROWS path.
FINE_BANDS = [
    (0, 2, "g"), (2, 4, "g"), (4, 7, "v"), (7, 10, "v"),
    (10, 11, "g"), (11, 12, "g"), (12, 13, "g"), (13, 14, "g"),
    (14, 15, "g"), (15, 16, "g"), (16, 17, "g"), (17, 18, "g"),
    (18, 19, "g"), (19, 20, "g"),
]


def _gp_mul(nc, out, a, b):
    """out = a * b on GPSIMD (plain TensorTensor: >=4D APs are BIR-legal,
    unlike ScalarTensorTensor which the BIR verifier caps at 3D)."""
    nc.gpsimd.tensor_tensor(out, a, b, OP.mult)


def _gp_sub(nc, out, a, b):
    """out = a - b on GPSIMD."""
    nc.gpsimd.tensor_tensor(out, a, b, OP.subtract)


def _gp_add(nc, out, a, b):
    nc.gpsimd.tensor_tensor(out, a, b, OP.add)


def emit_kernel(tc, pos_ap, out_ap, b_core, n_streams):
    """Emit the per-core program. pos: [b_core, 40] f32, out: [b_core, 400] f32."""
    nc = tc.nc
    m_total = b_core // P
    if STREAM_SIZES is not None:
        sizes = list(STREAM_SIZES)
        n_streams = len(sizes)
    else:
        sizes = [m_total // n_streams] * n_streams
    assert sum(sizes) == m_total
    offs = [0]
    for sz in sizes:
        offs.append(offs[-1] + sz)

    pos_r = pos_ap.rearrange("(p m) f -> p m f", p=P)
    out_r = out_ap.rearrange("(p m) f -> p m f", p=P)
    big_bufs = 2 if (ACTIVE <= 2 and max(sizes) < 32) else 1

    with (
        tc.tile_pool(name="pos", bufs=1) as pos_pool,
        tc.tile_pool(name="A", bufs=big_bufs) as a_pool,
        tc.tile_pool(name="cov", bufs=big_bufs) as cov_pool,
        tc.tile_pool(name="rect", bufs=1) as rect_pool,
        tc.tile_pool(name="grect", bufs=1) as grect_pool,
        tc.tile_pool(name="small", bufs=2) as small_pool,
    ):
        # prefetch every stream's positions up front
        pos_tiles = []
        for s in range(n_streams):
            pos_t = pos_pool.tile([P, sizes[s] * N * D], F32, tag=f"pos{s % 4}")
            nc.sync.dma_start(pos_t[:, :], pos_r[:, offs[s] : offs[s + 1], :])
            pos_tiles.append(pos_t)

        def stream_gen(s):
            par = s % ACTIVE
            Mg = sizes[s]
            posv = pos_tiles[s][:, :].rearrange(
                "p (m i d) -> p m i d", m=Mg, i=N
            )
            A = a_pool.tile([P, Mg * N * N], F32, tag=f"A{par}")
            A4 = A[:, :].rearrange("p (m i j) -> p m i j", m=Mg, i=N)
            Av = A[:, :].rearrange("p (m x) -> p m x", m=Mg)

            # ---- covariance build over the upper rect cover ----
            cov_eng = COV_ENG0 if (s == 0 and COV_ENG0) else COV_ENG
            cov_tail = []
            for ri, (r0, r1) in enumerate(RECTS):
                nr, ncl = r1 - r0, N - r0
                reg = A4[:, :, r0:r1, r0:]
                sub_a = (
                    (lambda o, a, b: _gp_sub(nc, o, a, b))
                    if cov_eng[ri] == "g"
                    else nc.vector.tensor_sub
                )  # "m": subs on DVE, add on GP
                add_a = (
                    (lambda o, a, b: _gp_add(nc, o, a, b))
                    if cov_eng[ri] in ("g", "m")
                    else nc.vector.tensor_add
                )
                if COV_MERGED:
                    # dd[m,i,j,:] = p[i,:] - p[j,:] in ONE sub (both
                    # coords), square on ACT, strided add -> d^2 in A.
                    pi = (
                        posv[:, :, r0:r1, :]
                        .unsqueeze(3)
                        .broadcast_to([P, Mg, nr, ncl, D])
                    )
                    pj = (
                        posv[:, :, r0:, :]
                        .unsqueeze(2)
                        .broadcast_to([P, Mg, nr, ncl, D])
                    )
                    dd = cov_pool.tile(
                        [P, Mg * nr * ncl * D], F32, tag=f"dd{par}"
                    )
                    ddv = dd[:, :].rearrange(
                        "p (m i j d) -> p m i j d", m=Mg, i=nr, j=ncl
                    )
                    sub_a(ddv, pi, pj)
                    nc.scalar.square(dd[:, :], dd[:, :])
                    add_a(reg, ddv[:, :, :, :, 0], ddv[:, :, :, :, 1])
                else:
                    # dx into A (in-place square), dy in an E-sized tmp;
                    # rows split between GPSIMD (top COV_GP_ROWS[ri]) and
                    # DVE for fine-grained engine balance
                    dy = cov_pool.tile(
                        [P, Mg * nr * ncl], F32, tag=f"dd{par}"
                    )
                    dyv = dy[:, :].rearrange(
                        "p (m i j) -> p m i j", m=Mg, i=nr
                    )
                    csplit = r0 + COV_GP_ROWS[ri]
                    for ceng, pa, pb in (("g", r0, csplit), ("v", csplit, r1)):
                        pn = pb - pa
                        if pn <= 0:
                            continue
                        regp = A4[:, :, pa:pb, r0:]
                        dyp = dyv[:, :, pa - r0 : pb - r0]
                        xi = (
                            posv[:, :, pa:pb, 0]
                            .unsqueeze(3)
                            .broadcast_to([P, Mg, pn, ncl])
                        )
                        xj = (
                            posv[:, :, r0:, 0]
                            .unsqueeze(2)
                            .broadcast_to([P, Mg, pn, ncl])
                        )
                        yi = (
                            posv[:, :, pa:pb, 1]
                            .unsqueeze(3)
                            .broadcast_to([P, Mg, pn, ncl])
                        )
                        yj = (
                            posv[:, :, r0:, 1]
                            .unsqueeze(2)
                            .broadcast_to([P, Mg, pn, ncl])
                        )
                        if ceng == "g":
                            _gp_sub(nc, regp, xi, xj)
                            nc.scalar.square(regp, regp)
                            _gp_sub(nc, dyp, yi, yj)
                            nc.scalar.square(dyp, dyp)
                            _gp_add(nc, regp, regp, dyp)
                        else:
                            nc.vector.tensor_sub(regp, xi, xj)
                            nc.scalar.square(regp, regp)
                            nc.vector.tensor_sub(dyp, yi, yj)
                            nc.scalar.square(dyp, dyp)
                            nc.vector.tensor_add(regp, regp, dyp)
                dseg = Av[:, :, r0 * (N + 1) : (r1 - 1) * (N + 1) + 1 : N + 1]
                if COV_PHASED is True or (COV_PHASED == 2 and ri > 0):
                    cov_tail.append((reg, dseg))
                else:
                    nc.scalar.sqrt(reg, reg)
                    nc.scalar.activation(reg, reg, AF.Exp, scale=-PHI)
                    # nugget on this rect's diagonal segment only: pivot k's
                    # reciprocal then depends on rect band(k) alone, letting
                    # early pivots overlap the remaining rects' cov chains
                    if TAU_ENG == "g":
                        nc.gpsimd.tensor_scalar_add(dseg, dseg, TAU)
                    elif TAU_ENG == "a":
                        nc.scalar.add(dseg, dseg, TAU)
                    else:
                        nc.vector.tensor_scalar_add(dseg, dseg, TAU)
                yield
            if cov_tail:
                for reg, _ in cov_tail:
                    nc.scalar.sqrt(reg, reg)
                for reg, _ in cov_tail:
                    nc.scalar.activation(reg, reg, AF.Exp, scale=-PHI)
                for _, dseg in cov_tail:
                    if TAU_ENG == "g":
                        nc.gpsimd.tensor_scalar_add(dseg, dseg, TAU)
                    elif TAU_ENG == "a":
                        nc.scalar.add(dseg, dseg, TAU)
                    else:
                        nc.vector.tensor_scalar_add(dseg, dseg, TAU)
                yield


            # ---- sweep all 20 pivots (gather-free) ----
            # The raw pivot column/row is read straight out of A as
            # broadcast operands of the rank-1 muls (column k for rows
            # above the pivot, row k for rows below); cr (= c * 1/pivot)
            # is computed from A the same way. All muls are emitted
            # before any sub so the in-place subs (which corrupt the
            # pivot row/col: cr[k] = 1) never race the raw reads.
            # The reciprocal for pivot k+1 is issued right after the sub
            # that finalizes A[k+1,k+1], hiding it under pivot k's tail.
            rK = small_pool.tile([P, Mg], F32, tag=f"r{par}")
            nc.vector.reciprocal(rK[:, :], A4[:, :, 0, 0])
            for k in range(N):
                crK = small_pool.tile([P, Mg * N], F32, tag=f"cr{par}")
                cr3 = crK[:, :].rearrange("p (m i) -> p m i", m=Mg)
                rb = rK[:, :].unsqueeze(2).broadcast_to([P, Mg, N])

                def crmul(which, o, a, b):
                    # "s": col part on GPSIMD; "s2": col part alternates
                    # engines by pivot parity (finer-grained balance)
                    if which == 0 and (
                        CR_ENG == "s" or (CR_ENG == "s2" and k % 2 == 0)
                    ):
                        _gp_mul(nc, o, a, b)
                    elif CR_ENG == "g":
                        _gp_mul(nc, o, a, b)
                    else:
                        nc.vector.tensor_mul(o, a, b)

                if k:
                    crmul(0, cr3[:, :, :k], A4[:, :, :k, k], rb[:, :, :k])
                crmul(1, cr3[:, :, k:], A4[:, :, k, k:], rb[:, :, k:])

                def c_raw(a, b):
                    """Broadcast AP of raw c[a:b] read from A's storage."""
                    if b <= k + 1:  # rows at or above the pivot: column k
                        return A4[:, :, a:b, k].unsqueeze(3)
                    return A4[:, :, k, a:b].unsqueeze(3)  # below: row k

                last = k == N - 1
                subs = []
                gp_rows = GP_ROWS_EVEN if k % 2 == 0 else GP_ROWS
                if FINE_BANDS is not None:
                    band_iter = [
                        (ri, r0, r1, eng, r0, r1)
                        for ri, (r0, r1, eng) in enumerate(FINE_BANDS)
                    ]
                else:
                    band_iter = []
                    for ri, (r0, r1) in enumerate(RECTS):
                        gsplit = r0 + gp_rows[ri]
                        band_iter.append((ri, r0, r1, "g", r0, gsplit))
                        band_iter.append((ri, r0, r1, "v", gsplit, r1))
                for ri, r0, r1, eng, a, b in band_iter:
                    ncl = N - r0
                    if True:
                        if b <= a:
                            continue
                        # rows to update: [a,b) minus the pivot row k
                        # (its results are discarded: the pivot row/col
                        # copies rewrite it below). Rows above the pivot
                        # read c from column k, rows below from row k.
                        if a <= k < b:
                            pieces = [(a, k), (k + 1, b)]
                        else:
                            pieces = [(a, b)]
                        pieces = [(pa, pb) for pa, pb in pieces if pb > pa]
                        if not pieces:
                            continue
                        tot = sum(pb - pa for pa, pb in pieces)
                        pool = grect_pool if eng == "g" else rect_pool
                        tmp = pool.tile(
                            [P, Mg * tot * ncl], F32, tag=f"t{eng}{par}r{ri}"
                        )
                        tv = tmp[:, :].rearrange(
                            "p (m i j) -> p m i j", m=Mg, i=tot
                        )
                        mul = (
                            (lambda o, x, y: _gp_mul(nc, o, x, y))
                            if eng == "g"
                            else nc.vector.tensor_mul
                        )
                        ofs = 0
                        for (pa, pb) in pieces:
                            pn = pb - pa
                            crb = (
                                cr3[:, :, r0:]
                                .unsqueeze(2)
                                .broadcast_to([P, Mg, pn, ncl])
                            )
                            mul(
                                tv[:, :, ofs : ofs + pn],
                                c_raw(pa, pb).broadcast_to([P, Mg, pn, ncl]),
                                crb,
                            )
                            # does this finalize next pivot's diagonal?
                            owns_next = pa <= k + 1 < pb
                            subs.append(
                                (
                                    eng,
                                    A4[:, :, pa:pb, r0:],
                                    tv[:, :, ofs : ofs + pn],
                                    owns_next,
                                )
                            )
                            ofs += pn
                subs.sort(key=lambda t: not t[3])  # next-diag owner first
                for si, (eng, reg, tv, owns_next) in enumerate(subs):
                    if eng == "g":
                        if last:
                            # fold the final negation into the last
                            # pivot: reg <- tv - reg = -(reg - tv)
                            _gp_sub(nc, reg, tv, reg)
                        else:
                            _gp_sub(nc, reg, reg, tv)
                    else:
                        if last:
                            nc.vector.tensor_sub(reg, tv, reg)
                        else:
                            nc.vector.tensor_sub(reg, reg, tv)
                    if owns_next and not last:
                        rK_next = small_pool.tile(
                            [P, Mg], F32, tag=f"r{par}"
                        )
                        nc.vector.reciprocal(
                            rK_next[:, :], A4[:, :, k + 1, k + 1]
                        )
                # pivot row/col (upper parts) <- cr; diag <- -r
                # (the last pivot writes negated values: the whole rect
                # cover holds -result after its reverse subtract)
                if k < N - 1:
                    pcopy = {
                        "v": nc.vector.tensor_copy,
                        "a": nc.scalar.copy,
                        "g": nc.gpsimd.tensor_copy,
                    }[PIVOT_COPY_ENG]
                    if k:
                        pcopy(A4[:, :, :k, k], cr3[:, :, :k])
                    pcopy(A4[:, :, k, k + 1 :], cr3[:, :, k + 1 :])
                    if DIAG_ENG == "a":
                        nc.scalar.mul(A4[:, :, k, k], rK[:, :], -1.0)
                    elif DIAG_ENG == "g":
                        nc.gpsimd.tensor_scalar_mul(
                            A4[:, :, k, k], rK[:, :], -1.0
                        )
                    else:
                        nc.vector.tensor_scalar_mul(
                            A4[:, :, k, k], rK[:, :], -1.0
                        )
                else:
                    if PIVOT_COPY_ENG == "a":
                        nc.scalar.mul(A4[:, :, :k, k], cr3[:, :, :k], -1.0)
                    else:
                        nc.vector.tensor_scalar_mul(
                            A4[:, :, :k, k], cr3[:, :, :k], -1.0
                        )
                    nc.vector.tensor_copy(A4[:, :, k, k], rK[:, :])
                if k < N - 1:
                    rK = rK_next
                yield

            # ---- finalize: mirror upper -> lower (values already negated),
            # in m-halves so the first half's store overlaps the second
            # half's mirror. The last stream has no concurrent work left,
            # so its mirror runs split across the otherwise-idle DVE+GPSIMD.
            tail = s == n_streams - 1
            h = Mg // 2
            for hi, (m0, m1) in enumerate(((0, h), (h, Mg))):
                for i in range(N - 1):
                    if tail:
                        mcopy = (
                            nc.vector.tensor_copy
                            if i % 2
                            else nc.gpsimd.tensor_copy
                        )
                    elif MIRROR_ENG == "a":
                        mcopy = nc.scalar.copy
                    else:
                        mcopy = nc.vector.tensor_copy
                    mcopy(
                        A4[:, m0:m1, i + 1 :, i], A4[:, m0:m1, i, i + 1 :]
                    )
                # the last stream's second half goes out via the ACT
                # HWDGE queue so both halves' transfers overlap (ACT is
                # idle during the pipeline drain; mid-run streams stay on
                # the SP queue where the issue cost is off-engine)
                dma_eng = nc.scalar if (tail and hi == 1) else nc.sync
                dma_eng.dma_start(
                    out_r[:, offs[s] + m0 : offs[s] + m1, :],
                    A[:, m0 * N * N : m1 * N * N],
                )
                yield

        pending = list(range(n_streams))
        active = [stream_gen(pending.pop(0))]
        if COV_PAR_START and pending:
            # run both initial streams' cov builds in parallel, then
            # prime stream 0's sweep so pivots stay phase-offset
            active.append(stream_gen(pending.pop(0)))
            for _ in range(len(RECTS)):
                for gen in active:
                    next(gen)
            for _ in range(STAGGER):
                next(active[0])
        else:
            # prime the first stream so concurrent streams stay offset
            for _ in range(STAGGER):
                next(active[0])
        while pending or active:
            while len(active) < ACTIVE and pending:
                active.append(stream_gen(pending.pop(0)))
            for gen in list(active):
                try:
                    next(gen)
                except StopIteration:
                    active.remove(gen)


_CACHE = {}


def build_nc(b_core=B_CORE, n_streams=None, num_devices=N_CORES):
    if n_streams is None:
        n_streams = N_STREAMS
    key = (b_core, n_streams, num_devices)
    if key in _CACHE:
        return _CACHE[key]
    nc = bacc.Bacc(
        "TRN2", target_bir_lowering=False, debug=False, num_devices=num_devices
    )
    pos_d = nc.dram_tensor("pos", [b_core, N * D], F32, kind="ExternalInput")
    out_d = nc.dram_tensor("out", [b_core, N * N], F32, kind="ExternalOutput")
    with tile.TileContext(nc) as tc:
        emit_kernel(tc, pos_d.ap(), out_d.ap(), b_core, n_streams)
    nc.compile()
    _CACHE[key] = nc
    return nc


def run(pos_full, b_core=B_CORE, n_streams=None, n_cores=N_CORES, **kw):
    """pos_full: [n_cores*b_core, 20, 2] f32 -> [n_cores*b_core, 20, 20] f32."""
    nc = build_nc(b_core, n_streams, n_cores)
    flat = np.ascontiguousarray(
        np.asarray(pos_full, dtype=np.float32).reshape(-1, N * D)
    )
    in_maps = [
        {"pos": flat[i * b_core : (i + 1) * b_core]} for i in range(n_cores)
    ]
    res = run_bass_kernel_spmd(nc, in_maps, core_ids=list(range(n_cores)), **kw)
    out = np.concatenate([r["out"] for r in res.results], axis=0)
    return out.reshape(-1, N, N), res


def kernel(neighbor_positions, edge_list=None):
    out, _ = run(neighbor_positions)
    return out



# revision 23
# speedup vs baseline: 1.1125x; 1.0010x over previous
"""Batched 20x20 SPD covariance-matrix inversion on 8 Trainium2 NeuronCores.

For each of 131072 batches: build C = exp(-1.5 * pairwise_dist(pos)) + 0.01*I
from 20 2-D points, return C^{-1}.

Strategy (per core, data-parallel over batch):
 - batch-major layout: each of 128 SBUF partitions holds Mg matrices' full
   20x20 (400 fp32) in the free dim; N_STREAMS independent streams.
 - symmetric sweep operator (Gauss-Jordan preserving symmetry): only the
   upper triangle is updated each pivot, covered by 4 row-band rectangles.
 - gather-free sweep: the raw pivot column/row is read straight out of A
   as broadcast operands of the rank-1 muls (all muls are emitted before
   any in-place sub so the subs never race those reads), cr = c/pivot is
   computed from A the same way, and the reciprocal of the NEXT pivot is
   issued right after the sub that finalizes its diagonal element. The
   pivot row itself is excluded from the update (its band splits around
   it): those results are discarded by the pivot row/col rewrite anyway.
 - engine split: the rank-1 updates (2 tensor-tensor passes per element)
   are split row-wise between DVE and GPSIMD plain tensor_tensor ops
   (ScalarTensorTensor is BIR-capped at 3D, so unusable here); ACT does
   the cov-build square/sqrt/exp, the pivot row/col writes, and the
   mirror of the upper triangle into the lower.
 - the final negation (sweep yields -A^{-1}) is folded into the last
   pivot's subtracts (reversed operands), so no extra negate pass runs.
 - ACTIVE streams are software-pipelined round-robin at pivot granularity:
   one stream's serial pivot prefix hides under the other's rank-1 work;
   cov builds and finalizes of adjacent streams overlap sweeps the same
   way. All pos DMAs are prefetched at kernel start; outputs are stored
   in m-halves so the first half's DMA overlaps the second's mirror.
"""

import numpy as np

import concourse.bass as bass  # noqa: F401  (registers engine APIs)
import concourse.tile as tile
from concourse import bacc, mybir
from concourse.bass_utils import run_bass_kernel_spmd

N = 20                  # matrix dim
D = 2                   # coord dim
PHI = 1.5
TAU = 0.01
P = 128                 # SBUF partitions
N_CORES = 8
B_TOTAL = 131072
B_CORE = B_TOTAL // N_CORES   # 16384

F32 = mybir.dt.float32
AF = mybir.ActivationFunctionType
OP = mybir.AluOpType

# Upper-triangle rectangle cover: rows [r0,r1) x cols [r0,N)
RECTS = [(0, 5), (5, 10), (10, 15), (15, 20)]

# --- engine-assignment knobs (autotuned via CoreSim) -----------------------
# per rect: how many of its rows (from the top) go to GPSIMD for the rank-1
# update; the rest go to DVE.
GP_ROWS = [5, 1, 1, 5]
# on even pivots one extra row per listed rect goes to GPSIMD (half-row
# granularity for the DVE/GPSIMD balance)
GP_ROWS_EVEN = [5, 1, 1, 5]
# per rect: cov-build tensor-tensor ops (dx, dy, add) engine: "v" DVE, "g" GP
COV_ENG = ["v", "g", "g", "v"]
# per rect: cov rows (from the top) built on GPSIMD; rest on DVE.
# [0,5,5,0] reproduces COV_ENG ["v","g","g","v"].
COV_GP_ROWS = [0, 5, 5, 2]
CR_ENG = "s2"            # cr = c * r:  "v" DVE tensor_mul, "g" GP stt
MIRROR_ENG = "a"        # "a" ACT copy(scale=-1) pre-negate, "v" DVE post
PIVOT_COPY_ENG = "a"    # pivot row/col <- cr copies: "v" DVE, "a" ACT, "g" GP
N_STREAMS = 8           # independent m-slices (Mg = B_CORE/P/N_STREAMS)
# per-stream m sizes (must sum to B_CORE/P); tapered tail shortens the
# final streams' drain while the pipeline is no longer full
STREAM_SIZES = [18, 18, 18, 18, 18, 14, 12, 12]
ACTIVE = 2              # streams pipelined concurrently
STAGGER = 6             # yields to prime stream 0 before starting stream 1
COV_MERGED = False       # cov: one 2-coord sub (needs 2E dd tile) vs dx/dy
DIAG_ENG = "a"          # per-pivot diag<- -r + TAU add: "a" ACT, "v" DVE, "g" GP
COV_PAR_START = False    # run both initial streams' cov builds in parallel
COV_ENG0 = None         # optional cov engine mix for stream 0 (startup ramp)
TAU_ENG = "g"           # per-rect diag nugget add: "v" DVE, "g" GP
# batch all rects' d^2 first, then all sqrts, then all exps: the ACT
# function table holds square+copy in every set but sqrt and exp live in
# different sets, so interleaving sqrt/exp per rect forces a 1283ns
# LoadActFuncSet per switch (~57 loads); phased order pays 2 per stream.
COV_PHASED = False
# Fine-grained rank-1 cover: list of (r0, r1, eng) bands, each updating
# rows [r0,r1) x cols [r0,N).  Narrow bands with their own column start
# carry far less sub-diagonal garbage than the 4x5 rect cover (whose
# lower rows update cols from the parent rect's r0).  GPSIMD tensor ops
# have no per-instruction engine overhead in the cost model, so it takes
# the narrow bottom bands; DVE (60ns init per instr) keeps wide top ones.
# Carve row k+1 out of a GPSIMD band into a dedicated DVE piece so the
# diag-finalizing sub and the next pivot's reciprocal stay on one engine
# (no cross-engine semaphore on the serial pivot chain).
OWNS_V = False
# None falls back to the baseline RECTS/GP_ROWS path.
FINE_BANDS = [
    (0, 2, "g"), (2, 4, "g"), (4, 7, "v"), (7, 10, "v"),
    (10, 11, "g"), (11, 12, "g"), (12, 13, "g"), (13, 14, "g"),
    (14, 15, "g"), (15, 16, "g"), (16, 17, "g"), (17, 18, "g"),
    (18, 19, "g"), (19, 20, "g"),
]


def _gp_mul(nc, out, a, b):
    """out = a * b on GPSIMD (plain TensorTensor: >=4D APs are BIR-legal,
    unlike ScalarTensorTensor which the BIR verifier caps at 3D)."""
    nc.gpsimd.tensor_tensor(out, a, b, OP.mult)


def _gp_sub(nc, out, a, b):
    """out = a - b on GPSIMD."""
    nc.gpsimd.tensor_tensor(out, a, b, OP.subtract)


def _gp_add(nc, out, a, b):
    nc.gpsimd.tensor_tensor(out, a, b, OP.add)


def emit_kernel(tc, pos_ap, out_ap, b_core, n_streams):
    """Emit the per-core program. pos: [b_core, 40] f32, out: [b_core, 400] f32."""
    nc = tc.nc
    m_total = b_core // P
    if STREAM_SIZES is not None:
        sizes = list(STREAM_SIZES)
        n_streams = len(sizes)
    else:
        sizes = [m_total // n_streams] * n_streams
    assert sum(sizes) == m_total
    offs = [0]
    for sz in sizes:
        offs.append(offs[-1] + sz)

    pos_r = pos_ap.rearrange("(p m) f -> p m f", p=P)
    out_r = out_ap.rearrange("(p m) f -> p m f", p=P)
    big_bufs = 2 if (ACTIVE <= 2 and max(sizes) < 32) else 1

    with (
        tc.tile_pool(name="pos", bufs=1) as pos_pool,
        tc.tile_pool(name="A", bufs=big_bufs) as a_pool,
        tc.tile_pool(name="cov", bufs=big_bufs) as cov_pool,
        tc.tile_pool(name="rect", bufs=1) as rect_pool,
        tc.tile_pool(name="grect", bufs=1) as grect_pool,
        tc.tile_pool(name="small", bufs=2) as small_pool,
    ):
        # prefetch every stream's positions up front
        pos_tiles = []
        for s in range(n_streams):
            pos_t = pos_pool.tile([P, sizes[s] * N * D], F32, tag=f"pos{s % 4}")
            nc.sync.dma_start(pos_t[:, :], pos_r[:, offs[s] : offs[s + 1], :])
            pos_tiles.append(pos_t)

        def stream_gen(s):
            par = s % ACTIVE
            Mg = sizes[s]
            posv = pos_tiles[s][:, :].rearrange(
                "p (m i d) -> p m i d", m=Mg, i=N
            )
            A = a_pool.tile([P, Mg * N * N], F32, tag=f"A{par}")
            A4 = A[:, :].rearrange("p (m i j) -> p m i j", m=Mg, i=N)
            Av = A[:, :].rearrange("p (m x) -> p m x", m=Mg)

            # ---- covariance build over the upper rect cover ----
            cov_eng = COV_ENG0 if (s == 0 and COV_ENG0) else COV_ENG
            cov_tail = []
            for ri, (r0, r1) in enumerate(RECTS):
                nr, ncl = r1 - r0, N - r0
                reg = A4[:, :, r0:r1, r0:]
                sub_a = (
                    (lambda o, a, b: _gp_sub(nc, o, a, b))
                    if cov_eng[ri] == "g"
                    else nc.vector.tensor_sub
                )  # "m": subs on DVE, add on GP
                add_a = (
                    (lambda o, a, b: _gp_add(nc, o, a, b))
                    if cov_eng[ri] in ("g", "m")
                    else nc.vector.tensor_add
                )
                if COV_MERGED:
                    # dd[m,i,j,:] = p[i,:] - p[j,:] in ONE sub (both
                    # coords), square on ACT, strided add -> d^2 in A.
                    pi = (
                        posv[:, :, r0:r1, :]
                        .unsqueeze(3)
                        .broadcast_to([P, Mg, nr, ncl, D])
                    )
                    pj = (
                        posv[:, :, r0:, :]
                        .unsqueeze(2)
                        .broadcast_to([P, Mg, nr, ncl, D])
                    )
                    dd = cov_pool.tile(
                        [P, Mg * nr * ncl * D], F32, tag=f"dd{par}"
                    )
                    ddv = dd[:, :].rearrange(
                        "p (m i j d) -> p m i j d", m=Mg, i=nr, j=ncl
                    )
                    sub_a(ddv, pi, pj)
                    nc.scalar.square(dd[:, :], dd[:, :])
                    add_a(reg, ddv[:, :, :, :, 0], ddv[:, :, :, :, 1])
                else:
                    # dx into A (in-place square), dy in an E-sized tmp;
                    # rows split between GPSIMD (top COV_GP_ROWS[ri]) and
                    # DVE for fine-grained engine balance
                    dy = cov_pool.tile(
                        [P, Mg * nr * ncl], F32, tag=f"dd{par}"
                    )
                    dyv = dy[:, :].rearrange(
                        "p (m i j) -> p m i j", m=Mg, i=nr
                    )
                    csplit = r0 + COV_GP_ROWS[ri]
                    for ceng, pa, pb in (("g", r0, csplit), ("v", csplit, r1)):
                        pn = pb - pa
                        if pn <= 0:
                            continue
                        regp = A4[:, :, pa:pb, r0:]
                        dyp = dyv[:, :, pa - r0 : pb - r0]
                        xi = (
                            posv[:, :, pa:pb, 0]
                            .unsqueeze(3)
                            .broadcast_to([P, Mg, pn, ncl])
                        )
                        xj = (
                            posv[:, :, r0:, 0]
                            .unsqueeze(2)
                            .broadcast_to([P, Mg, pn, ncl])
                        )
                        yi = (
                            posv[:, :, pa:pb, 1]
                            .unsqueeze(3)
                            .broadcast_to([P, Mg, pn, ncl])
                        )
                        yj = (
                            posv[:, :, r0:, 1]
                            .unsqueeze(2)
                            .broadcast_to([P, Mg, pn, ncl])
                        )
                        if ceng == "g":
                            _gp_sub(nc, regp, xi, xj)
                            nc.scalar.square(regp, regp)
                            _gp_sub(nc, dyp, yi, yj)
                            nc.scalar.square(dyp, dyp)
                            _gp_add(nc, regp, regp, dyp)
                        else:
                            nc.vector.tensor_sub(regp, xi, xj)
                            nc.scalar.square(regp, regp)
                            nc.vector.tensor_sub(dyp, yi, yj)
                            nc.scalar.square(dyp, dyp)
                            nc.vector.tensor_add(regp, regp, dyp)
                dseg = Av[:, :, r0 * (N + 1) : (r1 - 1) * (N + 1) + 1 : N + 1]
                if COV_PHASED is True or (COV_PHASED == 2 and ri > 0):
                    cov_tail.append((reg, dseg))
                else:
                    nc.scalar.sqrt(reg, reg)
                    nc.scalar.activation(reg, reg, AF.Exp, scale=-PHI)
                    # nugget on this rect's diagonal segment only: pivot k's
                    # reciprocal then depends on rect band(k) alone, letting
                    # early pivots overlap the remaining rects' cov chains
                    if TAU_ENG == "g":
                        nc.gpsimd.tensor_scalar_add(dseg, dseg, TAU)
                    elif TAU_ENG == "a":
                        nc.scalar.add(dseg, dseg, TAU)
                    else:
                        nc.vector.tensor_scalar_add(dseg, dseg, TAU)
                yield
            if cov_tail:
                for reg, _ in cov_tail:
                    nc.scalar.sqrt(reg, reg)
                for reg, _ in cov_tail:
                    nc.scalar.activation(reg, reg, AF.Exp, scale=-PHI)
                for _, dseg in cov_tail:
                    if TAU_ENG == "g":
                        nc.gpsimd.tensor_scalar_add(dseg, dseg, TAU)
                    elif TAU_ENG == "a":
                        nc.scalar.add(dseg, dseg, TAU)
                    else:
                        nc.vector.tensor_scalar_add(dseg, dseg, TAU)
                yield


            # ---- sweep all 20 pivots (gather-free) ----
            # The raw pivot column/row is read straight out of A as
            # broadcast operands of the rank-1 muls (column k for rows
            # above the pivot, row k for rows below); cr (= c * 1/pivot)
            # is computed from A the same way. All muls are emitted
            # before any sub so the in-place subs (which corrupt the
            # pivot row/col: cr[k] = 1) never race the raw reads.
            # The reciprocal for pivot k+1 is issued right after the sub
            # that finalizes A[k+1,k+1], hiding it under pivot k's tail.
            rK = small_pool.tile([P, Mg], F32, tag=f"r{par}")
            nc.vector.reciprocal(rK[:, :], A4[:, :, 0, 0])
            for k in range(N):
                crK = small_pool.tile([P, Mg * N], F32, tag=f"cr{par}")
                cr3 = crK[:, :].rearrange("p (m i) -> p m i", m=Mg)
                rb = rK[:, :].unsqueeze(2).broadcast_to([P, Mg, N])

                def crmul(which, o, a, b):
                    # "s": col part on GPSIMD; "s2": col part alternates
                    # engines by pivot parity; "r": row part on GPSIMD,
                    # col part on DVE; "r2": row part alternates
                    if which == 0 and (
                        CR_ENG == "s" or (CR_ENG == "s2" and k % 2 == 0)
                    ):
                        _gp_mul(nc, o, a, b)
                    elif which == 1 and (
                        CR_ENG == "r" or (CR_ENG == "r2" and k % 2 == 0)
                    ):
                        _gp_mul(nc, o, a, b)
                    elif CR_ENG == "g":
                        _gp_mul(nc, o, a, b)
                    else:
                        nc.vector.tensor_mul(o, a, b)

                if k:
                    crmul(0, cr3[:, :, :k], A4[:, :, :k, k], rb[:, :, :k])
                crmul(1, cr3[:, :, k:], A4[:, :, k, k:], rb[:, :, k:])

                def c_raw(a, b):
                    """Broadcast AP of raw c[a:b] read from A's storage."""
                    if b <= k + 1:  # rows at or above the pivot: column k
                        return A4[:, :, a:b, k].unsqueeze(3)
                    return A4[:, :, k, a:b].unsqueeze(3)  # below: row k

                last = k == N - 1
                subs = []
                gp_rows = GP_ROWS_EVEN if k % 2 == 0 else GP_ROWS
                if FINE_BANDS is not None:
                    band_iter = []
                    for ri, (r0, r1, eng) in enumerate(FINE_BANDS):
                        if (
                            OWNS_V
                            and eng == "g"
                            and r0 <= k + 1 < r1
                            and k < N - 1
                        ):
                            # dedicated DVE piece for the next-pivot row
                            if k + 2 < r1:
                                band_iter.append(
                                    (ri, r0, r1, "g", k + 2, r1)
                                )
                            if r0 < k + 1:
                                band_iter.append(
                                    (ri, r0, r1, "g", r0, k + 1)
                                )
                            band_iter.append(
                                (100 + ri, r0, r1, "v", k + 1, k + 2)
                            )
                        else:
                            band_iter.append((ri, r0, r1, eng, r0, r1))
                else:
                    band_iter = []
                    for ri, (r0, r1) in enumerate(RECTS):
                        gsplit = r0 + gp_rows[ri]
                        band_iter.append((ri, r0, r1, "g", r0, gsplit))
                        band_iter.append((ri, r0, r1, "v", gsplit, r1))
                for ri, r0, r1, eng, a, b in band_iter:
                    ncl = N - r0
                    if True:
                        if b <= a:
                            continue
                        # rows to update: [a,b) minus the pivot row k
                        # (its results are discarded: the pivot row/col
                        # copies rewrite it below). Rows above the pivot
                        # read c from column k, rows below from row k.
                        if a <= k < b:
                            pieces = [(a, k), (k + 1, b)]
                        else:
                            pieces = [(a, b)]
                        pieces = [(pa, pb) for pa, pb in pieces if pb > pa]
                        if not pieces:
                            continue
                        tot = sum(pb - pa for pa, pb in pieces)
                        pool = grect_pool if eng == "g" else rect_pool
                        tmp = pool.tile(
                            [P, Mg * tot * ncl], F32, tag=f"t{eng}{par}r{ri}"
                        )
                        tv = tmp[:, :].rearrange(
                            "p (m i j) -> p m i j", m=Mg, i=tot
                        )
                        mul = (
                            (lambda o, x, y: _gp_mul(nc, o, x, y))
                            if eng == "g"
                            else nc.vector.tensor_mul
                        )
                        ofs = 0
                        for (pa, pb) in pieces:
                            pn = pb - pa
                            crb = (
                                cr3[:, :, r0:]
                                .unsqueeze(2)
                                .broadcast_to([P, Mg, pn, ncl])
                            )
                            mul(
                                tv[:, :, ofs : ofs + pn],
                                c_raw(pa, pb).broadcast_to([P, Mg, pn, ncl]),
                                crb,
                            )
                            # does this finalize next pivot's diagonal?
                            owns_next = pa <= k + 1 < pb
                            subs.append(
                                (
                                    eng,
                                    A4[:, :, pa:pb, r0:],
                                    tv[:, :, ofs : ofs + pn],
                                    owns_next,
                                )
                            )
                            ofs += pn
                subs.sort(key=lambda t: not t[3])  # next-diag owner first
                for si, (eng, reg, tv, owns_next) in enumerate(subs):
                    if eng == "g":
                        if last:
                            # fold the final negation into the last
                            # pivot: reg <- tv - reg = -(reg - tv)
                            _gp_sub(nc, reg, tv, reg)
                        else:
                            _gp_sub(nc, reg, reg, tv)
                    else:
                        if last:
                            nc.vector.tensor_sub(reg, tv, reg)
                        else:
                            nc.vector.tensor_sub(reg, reg, tv)
                    if owns_next and not last:
                        rK_next = small_pool.tile(
                            [P, Mg], F32, tag=f"r{par}"
                        )
                        nc.vector.reciprocal(
                            rK_next[:, :], A4[:, :, k + 1, k + 1]
                        )
                # pivot row/col (upper parts) <- cr; diag <- -r
                # (the last pivot writes negated values: the whole rect
                # cover holds -result after its reverse subtract)
                if k < N - 1:
                    pcopy = {
                        "v": nc.vector.tensor_copy,
                        "a": nc.scalar.copy,
                        "g": nc.gpsimd.tensor_copy,
                    }[PIVOT_COPY_ENG]
                    if k:
                        pcopy(A4[:, :, :k, k], cr3[:, :, :k])
                    pcopy(A4[:, :, k, k + 1 :], cr3[:, :, k + 1 :])
                    if DIAG_ENG == "a":
                        nc.scalar.mul(A4[:, :, k, k], rK[:, :], -1.0)
                    elif DIAG_ENG == "g":
                        nc.gpsimd.tensor_scalar_mul(
                            A4[:, :, k, k], rK[:, :], -1.0
                        )
                    else:
                        nc.vector.tensor_scalar_mul(
                            A4[:, :, k, k], rK[:, :], -1.0
                        )
                else:
                    if PIVOT_COPY_ENG == "a":
                        nc.scalar.mul(A4[:, :, :k, k], cr3[:, :, :k], -1.0)
                    else:
                        nc.vector.tensor_scalar_mul(
                            A4[:, :, :k, k], cr3[:, :, :k], -1.0
                        )
                    nc.vector.tensor_copy(A4[:, :, k, k], rK[:, :])
                if k < N - 1:
                    rK = rK_next
                yield

            # ---- finalize: mirror upper -> lower (values already negated),
            # in m-halves so the first half's store overlaps the second
            # half's mirror. The last stream has no concurrent work left,
            # so its mirror runs split across the otherwise-idle DVE+GPSIMD.
            tail = s == n_streams - 1
            h = Mg // 2
            for hi, (m0, m1) in enumerate(((0, h), (h, Mg))):
                for i in range(N - 1):
                    if tail:
                        mcopy = (
                            nc.vector.tensor_copy
                            if i % 2
                            else nc.gpsimd.tensor_copy
                        )
                    elif MIRROR_ENG == "a":
                        mcopy = nc.scalar.copy
                    else:
                        mcopy = nc.vector.tensor_copy
                    mcopy(
                        A4[:, m0:m1, i + 1 :, i], A4[:, m0:m1, i, i + 1 :]
                    )
                # the last stream's second half goes out via the ACT
                # HWDGE queue so both halves' transfers overlap (ACT is
                # idle during the pipeline drain; mid-run streams stay on
                # the SP queue where the issue cost is off-engine)
                dma_eng = nc.scalar if (tail and hi == 1) else nc.sync
                dma_eng.dma_start(
                    out_r[:, offs[s] + m0 : offs[s] + m1, :],
                    A[:, m0 * N * N : m1 * N * N],
                )
                yield

        pending = list(range(n_streams))
        active = [stream_gen(pending.pop(0))]
        if COV_PAR_START and pending:
            # run both initial streams' cov builds in parallel, then
            # prime stream 0's sweep so pivots stay phase-offset
            active.append(stream_gen(pending.pop(0)))
            for _ in range(len(RECTS)):
                for gen in active:
                    next(gen)
            for _ in range(STAGGER):
                next(active[0])
        else:
            # prime the first stream so concurrent streams stay offset
            for _ in range(STAGGER):
                next(active[0])
        while pending or active:
            while len(active) < ACTIVE and pending:
                active.append(stream_gen(pending.pop(0)))
            for gen in list(active):
                try:
                    next(gen)
                except StopIteration:
                    active.remove(gen)


_CACHE = {}


def build_nc(b_core=B_CORE, n_streams=None, num_devices=N_CORES):
    if n_streams is None:
        n_streams = N_STREAMS
    key = (b_core, n_streams, num_devices)
    if key in _CACHE:
        return _CACHE[key]
    nc = bacc.Bacc(
        "TRN2", target_bir_lowering=False, debug=False, num_devices=num_devices
    )
    pos_d = nc.dram_tensor("pos", [b_core, N * D], F32, kind="ExternalInput")
    out_d = nc.dram_tensor("out", [b_core, N * N], F32, kind="ExternalOutput")
    with tile.TileContext(nc) as tc:
        emit_kernel(tc, pos_d.ap(), out_d.ap(), b_core, n_streams)
    nc.compile()
    _CACHE[key] = nc
    return nc


def run(pos_full, b_core=B_CORE, n_streams=None, n_cores=N_CORES, **kw):
    """pos_full: [n_cores*b_core, 20, 2] f32 -> [n_cores*b_core, 20, 20] f32."""
    nc = build_nc(b_core, n_streams, n_cores)
    flat = np.ascontiguousarray(
        np.asarray(pos_full, dtype=np.float32).reshape(-1, N * D)
    )
    in_maps = [
        {"pos": flat[i * b_core : (i + 1) * b_core]} for i in range(n_cores)
    ]
    res = run_bass_kernel_spmd(nc, in_maps, core_ids=list(range(n_cores)), **kw)
    out = np.concatenate([r["out"] for r in res.results], axis=0)
    return out.reshape(-1, N, N), res


def kernel(neighbor_positions, edge_list=None):
    out, _ = run(neighbor_positions)
    return out



# revision 28
# speedup vs baseline: 1.1157x; 1.0029x over previous
"""Batched 20x20 SPD covariance-matrix inversion on 8 Trainium2 NeuronCores.

For each of 131072 batches: build C = exp(-1.5 * pairwise_dist(pos)) + 0.01*I
from 20 2-D points, return C^{-1}.

Strategy (per core, data-parallel over batch):
 - batch-major layout: each of 128 SBUF partitions holds Mg matrices' full
   20x20 (400 fp32) in the free dim; N_STREAMS independent streams.
 - symmetric sweep operator (Gauss-Jordan preserving symmetry): only the
   upper triangle is updated each pivot, covered by 4 row-band rectangles.
 - gather-free sweep: the raw pivot column/row is read straight out of A
   as broadcast operands of the rank-1 muls (all muls are emitted before
   any in-place sub so the subs never race those reads), cr = c/pivot is
   computed from A the same way, and the reciprocal of the NEXT pivot is
   issued right after the sub that finalizes its diagonal element. The
   pivot row itself is excluded from the update (its band splits around
   it): those results are discarded by the pivot row/col rewrite anyway.
 - engine split: the rank-1 updates (2 tensor-tensor passes per element)
   are split row-wise between DVE and GPSIMD plain tensor_tensor ops
   (ScalarTensorTensor is BIR-capped at 3D, so unusable here); ACT does
   the cov-build square/sqrt/exp, the pivot row/col writes, and the
   mirror of the upper triangle into the lower.
 - the final negation (sweep yields -A^{-1}) is folded into the last
   pivot's subtracts (reversed operands), so no extra negate pass runs.
 - ACTIVE streams are software-pipelined round-robin at pivot granularity:
   one stream's serial pivot prefix hides under the other's rank-1 work;
   cov builds and finalizes of adjacent streams overlap sweeps the same
   way. All pos DMAs are prefetched at kernel start; outputs are stored
   in m-halves so the first half's DMA overlaps the second's mirror.
"""

import numpy as np

import concourse.bass as bass  # noqa: F401  (registers engine APIs)
import concourse.tile as tile
from concourse import bacc, mybir
from concourse.bass_utils import run_bass_kernel_spmd

N = 20                  # matrix dim
D = 2                   # coord dim
PHI = 1.5
TAU = 0.01
P = 128                 # SBUF partitions
N_CORES = 8
B_TOTAL = 131072
B_CORE = B_TOTAL // N_CORES   # 16384

F32 = mybir.dt.float32
AF = mybir.ActivationFunctionType
OP = mybir.AluOpType

# Upper-triangle rectangle cover: rows [r0,r1) x cols [r0,N)
RECTS = [(0, 5), (5, 10), (10, 15), (15, 20)]

# --- engine-assignment knobs (autotuned via CoreSim) -----------------------
# per rect: how many of its rows (from the top) go to GPSIMD for the rank-1
# update; the rest go to DVE.
GP_ROWS = [5, 1, 1, 5]
# on even pivots one extra row per listed rect goes to GPSIMD (half-row
# granularity for the DVE/GPSIMD balance)
GP_ROWS_EVEN = [5, 1, 1, 5]
# per rect: cov-build tensor-tensor ops (dx, dy, add) engine: "v" DVE, "g" GP
COV_ENG = ["v", "g", "g", "v"]
# per rect: cov rows (from the top) built on GPSIMD; rest on DVE.
# [0,5,5,0] reproduces COV_ENG ["v","g","g","v"].
COV_GP_ROWS = [0, 5, 5, 2]
CR_ENG = "s2"            # cr = c * r:  "v" DVE tensor_mul, "g" GP stt
MIRROR_ENG = "a"        # "a" ACT copy(scale=-1) pre-negate, "v" DVE post
PIVOT_COPY_ENG = "a"    # pivot row/col <- cr copies: "v" DVE, "a" ACT, "g" GP
N_STREAMS = 8           # independent m-slices (Mg = B_CORE/P/N_STREAMS)
# per-stream m sizes (must sum to B_CORE/P); tapered tail shortens the
# final streams' drain while the pipeline is no longer full
STREAM_SIZES = [18, 18, 18, 18, 18, 14, 12, 12]
ACTIVE = 2              # streams pipelined concurrently
STAGGER = 9             # yields to prime stream 0 before starting stream 1
COV_MERGED = False       # cov: one 2-coord sub (needs 2E dd tile) vs dx/dy
DIAG_ENG = "a"          # per-pivot diag<- -r + TAU add: "a" ACT, "v" DVE, "g" GP
COV_PAR_START = False    # run both initial streams' cov builds in parallel
COV_ENG0 = None         # optional cov engine mix for stream 0 (startup ramp)
TAU_ENG = "g"           # per-rect diag nugget add: "v" DVE, "g" GP
# batch all rects' d^2 first, then all sqrts, then all exps: the ACT
# function table holds square+copy in every set but sqrt and exp live in
# different sets, so interleaving sqrt/exp per rect forces a 1283ns
# LoadActFuncSet per switch (~57 loads); phased order pays 2 per stream.
COV_PHASED = False
# Fine-grained rank-1 cover: list of (r0, r1, eng) bands, each updating
# rows [r0,r1) x cols [r0,N).  Narrow bands with their own column start
# carry far less sub-diagonal garbage than the 4x5 rect cover (whose
# lower rows update cols from the parent rect's r0).  GPSIMD tensor ops
# have no per-instruction engine overhead in the cost model, so it takes
# the narrow bottom bands; DVE (60ns init per instr) keeps wide top ones.
# cov-build d^2 over the fine bands (own column starts) instead of the
# coarse rects; sqrt/exp per band grouped per rect slot.
COV_FINE = False
# Carve row k+1 out of a GPSIMD band into a dedicated DVE piece so the
# diag-finalizing sub and the next pivot's reciprocal stay on one engine
# (no cross-engine semaphore on the serial pivot chain).
OWNS_V = False
# None falls back to the baseline RECTS/GP_ROWS path.
FINE_BANDS = [
    (0, 2, "g"), (2, 4, "g"), (4, 7, "v"), (7, 10, "v"),
    (10, 11, "g"), (11, 12, "g"), (12, 13, "g"), (13, 14, "g"),
    (14, 15, "g"), (15, 16, "g"), (16, 17, "g"), (17, 18, "g"),
    (18, 19, "g"), (19, 20, "g"),
]


def _gp_mul(nc, out, a, b):
    """out = a * b on GPSIMD (plain TensorTensor: >=4D APs are BIR-legal,
    unlike ScalarTensorTensor which the BIR verifier caps at 3D)."""
    nc.gpsimd.tensor_tensor(out, a, b, OP.mult)


def _gp_sub(nc, out, a, b):
    """out = a - b on GPSIMD."""
    nc.gpsimd.tensor_tensor(out, a, b, OP.subtract)


def _gp_add(nc, out, a, b):
    nc.gpsimd.tensor_tensor(out, a, b, OP.add)


def emit_kernel(tc, pos_ap, out_ap, b_core, n_streams):
    """Emit the per-core program. pos: [b_core, 40] f32, out: [b_core, 400] f32."""
    nc = tc.nc
    m_total = b_core // P
    if STREAM_SIZES is not None:
        sizes = list(STREAM_SIZES)
        n_streams = len(sizes)
    else:
        sizes = [m_total // n_streams] * n_streams
    assert sum(sizes) == m_total
    offs = [0]
    for sz in sizes:
        offs.append(offs[-1] + sz)

    pos_r = pos_ap.rearrange("(p m) f -> p m f", p=P)
    out_r = out_ap.rearrange("(p m) f -> p m f", p=P)
    big_bufs = 2 if (ACTIVE <= 2 and max(sizes) < 32) else 1

    with (
        tc.tile_pool(name="pos", bufs=1) as pos_pool,
        tc.tile_pool(name="A", bufs=big_bufs) as a_pool,
        tc.tile_pool(name="cov", bufs=big_bufs) as cov_pool,
        tc.tile_pool(name="rect", bufs=1) as rect_pool,
        tc.tile_pool(name="grect", bufs=1) as grect_pool,
        tc.tile_pool(name="small", bufs=2) as small_pool,
    ):
        # prefetch every stream's positions up front
        pos_tiles = []
        for s in range(n_streams):
            pos_t = pos_pool.tile([P, sizes[s] * N * D], F32, tag=f"pos{s % 4}")
            nc.sync.dma_start(pos_t[:, :], pos_r[:, offs[s] : offs[s + 1], :])
            pos_tiles.append(pos_t)

        def stream_gen(s):
            par = s % ACTIVE
            Mg = sizes[s]
            posv = pos_tiles[s][:, :].rearrange(
                "p (m i d) -> p m i d", m=Mg, i=N
            )
            A = a_pool.tile([P, Mg * N * N], F32, tag=f"A{par}")
            A4 = A[:, :].rearrange("p (m i j) -> p m i j", m=Mg, i=N)
            Av = A[:, :].rearrange("p (m x) -> p m x", m=Mg)

            # ---- covariance build over the upper rect cover ----
            cov_eng = COV_ENG0 if (s == 0 and COV_ENG0) else COV_ENG
            cov_tail = []
            if COV_FINE and FINE_BANDS is not None:
                # build d^2 over the FINE bands (each with its own column
                # start, so almost no sub-diagonal garbage is computed);
                # sqrt/exp run per band, grouped per rect slot so the ACT
                # table still loads only twice per rect.
                for ri, (r0, r1) in enumerate(RECTS):
                    here = [b for b in FINE_BANDS if r0 <= b[0] < r1]
                    for (b0, b1, beng) in here:
                        bn, bc = b1 - b0, N - b0
                        breg = A4[:, :, b0:b1, b0:]
                        dy = cov_pool.tile(
                            [P, Mg * bn * bc], F32, tag=f"dd{par}"
                        )
                        dyv = dy[:, :].rearrange(
                            "p (m i j) -> p m i j", m=Mg, i=bn
                        )
                        xi = (
                            posv[:, :, b0:b1, 0]
                            .unsqueeze(3)
                            .broadcast_to([P, Mg, bn, bc])
                        )
                        xj = (
                            posv[:, :, b0:, 0]
                            .unsqueeze(2)
                            .broadcast_to([P, Mg, bn, bc])
                        )
                        yi = (
                            posv[:, :, b0:b1, 1]
                            .unsqueeze(3)
                            .broadcast_to([P, Mg, bn, bc])
                        )
                        yj = (
                            posv[:, :, b0:, 1]
                            .unsqueeze(2)
                            .broadcast_to([P, Mg, bn, bc])
                        )
                        if beng == "v":
                            nc.vector.tensor_sub(breg, xi, xj)
                            nc.scalar.square(breg, breg)
                            nc.vector.tensor_sub(dyv, yi, yj)
                            nc.scalar.square(dyv, dyv)
                            nc.vector.tensor_add(breg, breg, dyv)
                        else:
                            _gp_sub(nc, breg, xi, xj)
                            nc.scalar.square(breg, breg)
                            _gp_sub(nc, dyv, yi, yj)
                            nc.scalar.square(dyv, dyv)
                            _gp_add(nc, breg, breg, dyv)
                    for (b0, b1, beng) in here:
                        nc.scalar.sqrt(A4[:, :, b0:b1, b0:], A4[:, :, b0:b1, b0:])
                    for (b0, b1, beng) in here:
                        nc.scalar.activation(
                            A4[:, :, b0:b1, b0:], A4[:, :, b0:b1, b0:],
                            AF.Exp, scale=-PHI,
                        )
                    dseg = Av[
                        :, :, r0 * (N + 1) : (r1 - 1) * (N + 1) + 1 : N + 1
                    ]
                    if TAU_ENG == "g":
                        nc.gpsimd.tensor_scalar_add(dseg, dseg, TAU)
                    else:
                        nc.vector.tensor_scalar_add(dseg, dseg, TAU)
                    yield
            for ri, (r0, r1) in enumerate(
                [] if (COV_FINE and FINE_BANDS is not None) else RECTS
            ):
                nr, ncl = r1 - r0, N - r0
                reg = A4[:, :, r0:r1, r0:]
                sub_a = (
                    (lambda o, a, b: _gp_sub(nc, o, a, b))
                    if cov_eng[ri] == "g"
                    else nc.vector.tensor_sub
                )  # "m": subs on DVE, add on GP
                add_a = (
                    (lambda o, a, b: _gp_add(nc, o, a, b))
                    if cov_eng[ri] in ("g", "m")
                    else nc.vector.tensor_add
                )
                if COV_MERGED:
                    # dd[m,i,j,:] = p[i,:] - p[j,:] in ONE sub (both
                    # coords), square on ACT, strided add -> d^2 in A.
                    pi = (
                        posv[:, :, r0:r1, :]
                        .unsqueeze(3)
                        .broadcast_to([P, Mg, nr, ncl, D])
                    )
                    pj = (
                        posv[:, :, r0:, :]
                        .unsqueeze(2)
                        .broadcast_to([P, Mg, nr, ncl, D])
                    )
                    dd = cov_pool.tile(
                        [P, Mg * nr * ncl * D], F32, tag=f"dd{par}"
                    )
                    ddv = dd[:, :].rearrange(
                        "p (m i j d) -> p m i j d", m=Mg, i=nr, j=ncl
                    )
                    sub_a(ddv, pi, pj)
                    nc.scalar.square(dd[:, :], dd[:, :])
                    add_a(reg, ddv[:, :, :, :, 0], ddv[:, :, :, :, 1])
                else:
                    # dx into A (in-place square), dy in an E-sized tmp;
                    # rows split between GPSIMD (top COV_GP_ROWS[ri]) and
                    # DVE for fine-grained engine balance
                    dy = cov_pool.tile(
                        [P, Mg * nr * ncl], F32, tag=f"dd{par}"
                    )
                    dyv = dy[:, :].rearrange(
                        "p (m i j) -> p m i j", m=Mg, i=nr
                    )
                    csplit = r0 + COV_GP_ROWS[ri]
                    for ceng, pa, pb in (("g", r0, csplit), ("v", csplit, r1)):
                        pn = pb - pa
                        if pn <= 0:
                            continue
                        regp = A4[:, :, pa:pb, r0:]
                        dyp = dyv[:, :, pa - r0 : pb - r0]
                        xi = (
                            posv[:, :, pa:pb, 0]
                            .unsqueeze(3)
                            .broadcast_to([P, Mg, pn, ncl])
                        )
                        xj = (
                            posv[:, :, r0:, 0]
                            .unsqueeze(2)
                            .broadcast_to([P, Mg, pn, ncl])
                        )
                        yi = (
                            posv[:, :, pa:pb, 1]
                            .unsqueeze(3)
                            .broadcast_to([P, Mg, pn, ncl])
                        )
                        yj = (
                            posv[:, :, r0:, 1]
                            .unsqueeze(2)
                            .broadcast_to([P, Mg, pn, ncl])
                        )
                        if ceng == "g":
                            _gp_sub(nc, regp, xi, xj)
                            nc.scalar.square(regp, regp)
                            _gp_sub(nc, dyp, yi, yj)
                            nc.scalar.square(dyp, dyp)
                            _gp_add(nc, regp, regp, dyp)
                        else:
                            nc.vector.tensor_sub(regp, xi, xj)
                            nc.scalar.square(regp, regp)
                            nc.vector.tensor_sub(dyp, yi, yj)
                            nc.scalar.square(dyp, dyp)
                            nc.vector.tensor_add(regp, regp, dyp)
                dseg = Av[:, :, r0 * (N + 1) : (r1 - 1) * (N + 1) + 1 : N + 1]
                if COV_PHASED is True or (COV_PHASED == 2 and ri > 0):
                    cov_tail.append((reg, dseg))
                else:
                    nc.scalar.sqrt(reg, reg)
                    nc.scalar.activation(reg, reg, AF.Exp, scale=-PHI)
                    # nugget on this rect's diagonal segment only: pivot k's
                    # reciprocal then depends on rect band(k) alone, letting
                    # early pivots overlap the remaining rects' cov chains
                    if TAU_ENG == "g":
                        nc.gpsimd.tensor_scalar_add(dseg, dseg, TAU)
                    elif TAU_ENG == "a":
                        nc.scalar.add(dseg, dseg, TAU)
                    else:
                        nc.vector.tensor_scalar_add(dseg, dseg, TAU)
                yield
            if cov_tail:
                for reg, _ in cov_tail:
                    nc.scalar.sqrt(reg, reg)
                for reg, _ in cov_tail:
                    nc.scalar.activation(reg, reg, AF.Exp, scale=-PHI)
                for _, dseg in cov_tail:
                    if TAU_ENG == "g":
                        nc.gpsimd.tensor_scalar_add(dseg, dseg, TAU)
                    elif TAU_ENG == "a":
                        nc.scalar.add(dseg, dseg, TAU)
                    else:
                        nc.vector.tensor_scalar_add(dseg, dseg, TAU)
                yield


            # ---- sweep all 20 pivots (gather-free) ----
            # The raw pivot column/row is read straight out of A as
            # broadcast operands of the rank-1 muls (column k for rows
            # above the pivot, row k for rows below); cr (= c * 1/pivot)
            # is computed from A the same way. All muls are emitted
            # before any sub so the in-place subs (which corrupt the
            # pivot row/col: cr[k] = 1) never race the raw reads.
            # The reciprocal for pivot k+1 is issued right after the sub
            # that finalizes A[k+1,k+1], hiding it under pivot k's tail.
            rK = small_pool.tile([P, Mg], F32, tag=f"r{par}")
            nc.vector.reciprocal(rK[:, :], A4[:, :, 0, 0])
            for k in range(N):
                crK = small_pool.tile([P, Mg * N], F32, tag=f"cr{par}")
                cr3 = crK[:, :].rearrange("p (m i) -> p m i", m=Mg)
                rb = rK[:, :].unsqueeze(2).broadcast_to([P, Mg, N])

                def crmul(which, o, a, b):
                    # "s": col part on GPSIMD; "s2": col part alternates
                    # engines by pivot parity; "r": row part on GPSIMD,
                    # col part on DVE; "r2": row part alternates
                    if which == 0 and (
                        CR_ENG == "s" or (CR_ENG == "s2" and k % 2 == 0)
                    ):
                        _gp_mul(nc, o, a, b)
                    elif which == 1 and (
                        CR_ENG == "r" or (CR_ENG == "r2" and k % 2 == 0)
                    ):
                        _gp_mul(nc, o, a, b)
                    elif CR_ENG == "g":
                        _gp_mul(nc, o, a, b)
                    else:
                        nc.vector.tensor_mul(o, a, b)

                if k:
                    crmul(0, cr3[:, :, :k], A4[:, :, :k, k], rb[:, :, :k])
                crmul(1, cr3[:, :, k:], A4[:, :, k, k:], rb[:, :, k:])

                def c_raw(a, b):
                    """Broadcast AP of raw c[a:b] read from A's storage."""
                    if b <= k + 1:  # rows at or above the pivot: column k
                        return A4[:, :, a:b, k].unsqueeze(3)
                    return A4[:, :, k, a:b].unsqueeze(3)  # below: row k

                last = k == N - 1
                subs = []
                gp_rows = GP_ROWS_EVEN if k % 2 == 0 else GP_ROWS
                if FINE_BANDS is not None:
                    band_iter = []
                    for ri, (r0, r1, eng) in enumerate(FINE_BANDS):
                        if eng == "a":  # alternate by pivot+stream parity
                            eng = "v" if (k + s) % 2 else "g"
                        elif eng == "A":
                            eng = "g" if (k + s) % 2 else "v"
                        if (
                            OWNS_V
                            and eng == "g"
                            and r0 <= k + 1 < r1
                            and k < N - 1
                        ):
                            # dedicated DVE piece for the next-pivot row
                            if k + 2 < r1:
                                band_iter.append(
                                    (ri, r0, r1, "g", k + 2, r1)
                                )
                            if r0 < k + 1:
                                band_iter.append(
                                    (ri, r0, r1, "g", r0, k + 1)
                                )
                            band_iter.append(
                                (100 + ri, r0, r1, "v", k + 1, k + 2)
                            )
                        else:
                            band_iter.append((ri, r0, r1, eng, r0, r1))
                else:
                    band_iter = []
                    for ri, (r0, r1) in enumerate(RECTS):
                        gsplit = r0 + gp_rows[ri]
                        band_iter.append((ri, r0, r1, "g", r0, gsplit))
                        band_iter.append((ri, r0, r1, "v", gsplit, r1))
                for ri, r0, r1, eng, a, b in band_iter:
                    ncl = N - r0
                    if True:
                        if b <= a:
                            continue
                        # rows to update: [a,b) minus the pivot row k
                        # (its results are discarded: the pivot row/col
                        # copies rewrite it below). Rows above the pivot
                        # read c from column k, rows below from row k.
                        if a <= k < b:
                            pieces = [(a, k), (k + 1, b)]
                        else:
                            pieces = [(a, b)]
                        pieces = [(pa, pb) for pa, pb in pieces if pb > pa]
                        if not pieces:
                            continue
                        tot = sum(pb - pa for pa, pb in pieces)
                        pool = grect_pool if eng == "g" else rect_pool
                        tmp = pool.tile(
                            [P, Mg * tot * ncl], F32, tag=f"t{eng}{par}r{ri}"
                        )
                        tv = tmp[:, :].rearrange(
                            "p (m i j) -> p m i j", m=Mg, i=tot
                        )
                        mul = (
                            (lambda o, x, y: _gp_mul(nc, o, x, y))
                            if eng == "g"
                            else nc.vector.tensor_mul
                        )
                        ofs = 0
                        for (pa, pb) in pieces:
                            pn = pb - pa
                            crb = (
                                cr3[:, :, r0:]
                                .unsqueeze(2)
                                .broadcast_to([P, Mg, pn, ncl])
                            )
                            mul(
                                tv[:, :, ofs : ofs + pn],
                                c_raw(pa, pb).broadcast_to([P, Mg, pn, ncl]),
                                crb,
                            )
                            # does this finalize next pivot's diagonal?
                            owns_next = pa <= k + 1 < pb
                            subs.append(
                                (
                                    eng,
                                    A4[:, :, pa:pb, r0:],
                                    tv[:, :, ofs : ofs + pn],
                                    owns_next,
                                )
                            )
                            ofs += pn
                subs.sort(key=lambda t: not t[3])  # next-diag owner first
                for si, (eng, reg, tv, owns_next) in enumerate(subs):
                    if eng == "g":
                        if last:
                            # fold the final negation into the last
                            # pivot: reg <- tv - reg = -(reg - tv)
                            _gp_sub(nc, reg, tv, reg)
                        else:
                            _gp_sub(nc, reg, reg, tv)
                    else:
                        if last:
                            nc.vector.tensor_sub(reg, tv, reg)
                        else:
                            nc.vector.tensor_sub(reg, reg, tv)
                    if owns_next and not last:
                        rK_next = small_pool.tile(
                            [P, Mg], F32, tag=f"r{par}"
                        )
                        nc.vector.reciprocal(
                            rK_next[:, :], A4[:, :, k + 1, k + 1]
                        )
                # pivot row/col (upper parts) <- cr; diag <- -r
                # (the last pivot writes negated values: the whole rect
                # cover holds -result after its reverse subtract)
                if k < N - 1:
                    pcopy = {
                        "v": nc.vector.tensor_copy,
                        "a": nc.scalar.copy,
                        "g": nc.gpsimd.tensor_copy,
                    }[PIVOT_COPY_ENG]
                    if k:
                        pcopy(A4[:, :, :k, k], cr3[:, :, :k])
                    pcopy(A4[:, :, k, k + 1 :], cr3[:, :, k + 1 :])
                    if DIAG_ENG == "a":
                        nc.scalar.mul(A4[:, :, k, k], rK[:, :], -1.0)
                    elif DIAG_ENG == "g":
                        nc.gpsimd.tensor_scalar_mul(
                            A4[:, :, k, k], rK[:, :], -1.0
                        )
                    else:
                        nc.vector.tensor_scalar_mul(
                            A4[:, :, k, k], rK[:, :], -1.0
                        )
                else:
                    if PIVOT_COPY_ENG == "a":
                        nc.scalar.mul(A4[:, :, :k, k], cr3[:, :, :k], -1.0)
                    else:
                        nc.vector.tensor_scalar_mul(
                            A4[:, :, :k, k], cr3[:, :, :k], -1.0
                        )
                    nc.vector.tensor_copy(A4[:, :, k, k], rK[:, :])
                if k < N - 1:
                    rK = rK_next
                yield

            # ---- finalize: mirror upper -> lower (values already negated),
            # in m-halves so the first half's store overlaps the second
            # half's mirror. The last stream has no concurrent work left,
            # so its mirror runs split across the otherwise-idle DVE+GPSIMD.
            tail = s == n_streams - 1
            h = Mg // 2
            for hi, (m0, m1) in enumerate(((0, h), (h, Mg))):
                for i in range(N - 1):
                    if tail:
                        mcopy = (
                            nc.vector.tensor_copy
                            if i % 2
                            else nc.gpsimd.tensor_copy
                        )
                    elif MIRROR_ENG == "a":
                        mcopy = nc.scalar.copy
                    else:
                        mcopy = nc.vector.tensor_copy
                    mcopy(
                        A4[:, m0:m1, i + 1 :, i], A4[:, m0:m1, i, i + 1 :]
                    )
                # the last stream's second half goes out via the ACT
                # HWDGE queue so both halves' transfers overlap (ACT is
                # idle during the pipeline drain; mid-run streams stay on
                # the SP queue where the issue cost is off-engine)
                dma_eng = nc.scalar if (tail and hi == 1) else nc.sync
                dma_eng.dma_start(
                    out_r[:, offs[s] + m0 : offs[s] + m1, :],
                    A[:, m0 * N * N : m1 * N * N],
                )
                yield

        pending = list(range(n_streams))
        active = [stream_gen(pending.pop(0))]
        if COV_PAR_START and pending:
            # run both initial streams' cov builds in parallel, then
            # prime stream 0's sweep so pivots stay phase-offset
            active.append(stream_gen(pending.pop(0)))
            for _ in range(len(RECTS)):
                for gen in active:
                    next(gen)
            for _ in range(STAGGER):
                next(active[0])
        else:
            # prime the first stream so concurrent streams stay offset
            for _ in range(STAGGER):
                next(active[0])
        while pending or active:
            while len(active) < ACTIVE and pending:
                active.append(stream_gen(pending.pop(0)))
            for gen in list(active):
                try:
                    next(gen)
                except StopIteration:
                    active.remove(gen)


_CACHE = {}


def build_nc(b_core=B_CORE, n_streams=None, num_devices=N_CORES):
    if n_streams is None:
        n_streams = N_STREAMS
    key = (b_core, n_streams, num_devices)
    if key in _CACHE:
        return _CACHE[key]
    nc = bacc.Bacc(
        "TRN2", target_bir_lowering=False, debug=False, num_devices=num_devices
    )
    pos_d = nc.dram_tensor("pos", [b_core, N * D], F32, kind="ExternalInput")
    out_d = nc.dram_tensor("out", [b_core, N * N], F32, kind="ExternalOutput")
    with tile.TileContext(nc) as tc:
        emit_kernel(tc, pos_d.ap(), out_d.ap(), b_core, n_streams)
    nc.compile()
    _CACHE[key] = nc
    return nc


def run(pos_full, b_core=B_CORE, n_streams=None, n_cores=N_CORES, **kw):
    """pos_full: [n_cores*b_core, 20, 2] f32 -> [n_cores*b_core, 20, 20] f32."""
    nc = build_nc(b_core, n_streams, n_cores)
    flat = np.ascontiguousarray(
        np.asarray(pos_full, dtype=np.float32).reshape(-1, N * D)
    )
    in_maps = [
        {"pos": flat[i * b_core : (i + 1) * b_core]} for i in range(n_cores)
    ]
    res = run_bass_kernel_spmd(nc, in_maps, core_ids=list(range(n_cores)), **kw)
    out = np.concatenate([r["out"] for r in res.results], axis=0)
    return out.reshape(-1, N, N), res


def kernel(neighbor_positions, edge_list=None):
    out, _ = run(neighbor_positions)
    return out



# revision 30
# speedup vs baseline: 1.1192x; 1.0032x over previous
"""Batched 20x20 SPD covariance-matrix inversion on 8 Trainium2 NeuronCores.

For each of 131072 batches: build C = exp(-1.5 * pairwise_dist(pos)) + 0.01*I
from 20 2-D points, return C^{-1}.

Strategy (per core, data-parallel over batch):
 - batch-major layout: each of 128 SBUF partitions holds Mg matrices' full
   20x20 (400 fp32) in the free dim; N_STREAMS independent streams.
 - symmetric sweep operator (Gauss-Jordan preserving symmetry): only the
   upper triangle is updated each pivot, covered by 4 row-band rectangles.
 - gather-free sweep: the raw pivot column/row is read straight out of A
   as broadcast operands of the rank-1 muls (all muls are emitted before
   any in-place sub so the subs never race those reads), cr = c/pivot is
   computed from A the same way, and the reciprocal of the NEXT pivot is
   issued right after the sub that finalizes its diagonal element. The
   pivot row itself is excluded from the update (its band splits around
   it): those results are discarded by the pivot row/col rewrite anyway.
 - engine split: the rank-1 updates (2 tensor-tensor passes per element)
   are split row-wise between DVE and GPSIMD plain tensor_tensor ops
   (ScalarTensorTensor is BIR-capped at 3D, so unusable here); ACT does
   the cov-build square/sqrt/exp, the pivot row/col writes, and the
   mirror of the upper triangle into the lower.
 - the final negation (sweep yields -A^{-1}) is folded into the last
   pivot's subtracts (reversed operands), so no extra negate pass runs.
 - ACTIVE streams are software-pipelined round-robin at pivot granularity:
   one stream's serial pivot prefix hides under the other's rank-1 work;
   cov builds and finalizes of adjacent streams overlap sweeps the same
   way. All pos DMAs are prefetched at kernel start; outputs are stored
   in m-halves so the first half's DMA overlaps the second's mirror.
"""

import numpy as np

import concourse.bass as bass  # noqa: F401  (registers engine APIs)
import concourse.tile as tile
from concourse import bacc, mybir
from concourse.bass_utils import run_bass_kernel_spmd

N = 20                  # matrix dim
D = 2                   # coord dim
PHI = 1.5
TAU = 0.01
P = 128                 # SBUF partitions
N_CORES = 8
B_TOTAL = 131072
B_CORE = B_TOTAL // N_CORES   # 16384

F32 = mybir.dt.float32
AF = mybir.ActivationFunctionType
OP = mybir.AluOpType

# Upper-triangle rectangle cover: rows [r0,r1) x cols [r0,N)
RECTS = [(0, 5), (5, 10), (10, 15), (15, 20)]

# --- engine-assignment knobs (autotuned via CoreSim) -----------------------
# per rect: how many of its rows (from the top) go to GPSIMD for the rank-1
# update; the rest go to DVE.
GP_ROWS = [5, 1, 1, 5]
# on even pivots one extra row per listed rect goes to GPSIMD (half-row
# granularity for the DVE/GPSIMD balance)
GP_ROWS_EVEN = [5, 1, 1, 5]
# per rect: cov-build tensor-tensor ops (dx, dy, add) engine: "v" DVE, "g" GP
COV_ENG = ["v", "g", "g", "v"]
# per rect: cov rows (from the top) built on GPSIMD; rest on DVE.
# [0,5,5,0] reproduces COV_ENG ["v","g","g","v"].
COV_GP_ROWS = [0, 5, 5, 2]
CR_ENG = "s2"            # cr = c * r:  "v" DVE tensor_mul, "g" GP stt
MIRROR_ENG = "a"        # "a" ACT copy(scale=-1) pre-negate, "v" DVE post
PIVOT_COPY_ENG = "a"    # pivot row/col <- cr copies: "v" DVE, "a" ACT, "g" GP
N_STREAMS = 8           # independent m-slices (Mg = B_CORE/P/N_STREAMS)
# per-stream m sizes (must sum to B_CORE/P); medium ramp-in streams, big
# middle, small tail to shorten the drain once no partner stream remains
STREAM_SIZES = [16, 16, 18, 18, 18, 18, 12, 12]
ACTIVE = 2              # streams pipelined concurrently
STAGGER = 9             # yields to prime stream 0 before starting stream 1
COV_MERGED = False       # cov: one 2-coord sub (needs 2E dd tile) vs dx/dy
DIAG_ENG = "a"          # per-pivot diag<- -r + TAU add: "a" ACT, "v" DVE, "g" GP
COV_PAR_START = False    # run both initial streams' cov builds in parallel
COV_ENG0 = None         # optional cov engine mix for stream 0 (startup ramp)
TAU_ENG = "g"           # per-rect diag nugget add: "v" DVE, "g" GP
# batch all rects' d^2 first, then all sqrts, then all exps: the ACT
# function table holds square+copy in every set but sqrt and exp live in
# different sets, so interleaving sqrt/exp per rect forces a 1283ns
# LoadActFuncSet per switch (~57 loads); phased order pays 2 per stream.
COV_PHASED = False
# Fine-grained rank-1 cover: list of (r0, r1, eng) bands, each updating
# rows [r0,r1) x cols [r0,N).  Narrow bands with their own column start
# carry far less sub-diagonal garbage than the 4x5 rect cover (whose
# lower rows update cols from the parent rect's r0).  GPSIMD tensor ops
# have no per-instruction engine overhead in the cost model, so it takes
# the narrow bottom bands; DVE (60ns init per instr) keeps wide top ones.
# cov-build d^2 over the fine bands (own column starts) instead of the
# coarse rects; sqrt/exp per band grouped per rect slot.
COV_FINE = False
# Carve row k+1 out of a GPSIMD band into a dedicated DVE piece so the
# diag-finalizing sub and the next pivot's reciprocal stay on one engine
# (no cross-engine semaphore on the serial pivot chain).
OWNS_V = False
# None falls back to the baseline RECTS/GP_ROWS path.
FINE_BANDS = [
    (0, 1, "g"), (1, 2, "g"), (2, 3, "g"), (3, 4, "g"),
    (4, 7, "v"), (7, 10, "v"),
    (10, 11, "g"), (11, 12, "g"), (12, 13, "g"), (13, 14, "g"),
    (14, 15, "g"), (15, 16, "g"), (16, 17, "g"), (17, 18, "g"),
    (18, 19, "g"), (19, 20, "g"),
]


def _gp_mul(nc, out, a, b):
    """out = a * b on GPSIMD (plain TensorTensor: >=4D APs are BIR-legal,
    unlike ScalarTensorTensor which the BIR verifier caps at 3D)."""
    nc.gpsimd.tensor_tensor(out, a, b, OP.mult)


def _gp_sub(nc, out, a, b):
    """out = a - b on GPSIMD."""
    nc.gpsimd.tensor_tensor(out, a, b, OP.subtract)


def _gp_add(nc, out, a, b):
    nc.gpsimd.tensor_tensor(out, a, b, OP.add)


def emit_kernel(tc, pos_ap, out_ap, b_core, n_streams):
    """Emit the per-core program. pos: [b_core, 40] f32, out: [b_core, 400] f32."""
    nc = tc.nc
    m_total = b_core // P
    if STREAM_SIZES is not None:
        sizes = list(STREAM_SIZES)
        n_streams = len(sizes)
    else:
        sizes = [m_total // n_streams] * n_streams
    assert sum(sizes) == m_total
    offs = [0]
    for sz in sizes:
        offs.append(offs[-1] + sz)

    pos_r = pos_ap.rearrange("(p m) f -> p m f", p=P)
    out_r = out_ap.rearrange("(p m) f -> p m f", p=P)
    big_bufs = 2 if (ACTIVE <= 2 and max(sizes) < 32) else 1

    with (
        tc.tile_pool(name="pos", bufs=1) as pos_pool,
        tc.tile_pool(name="A", bufs=big_bufs) as a_pool,
        tc.tile_pool(name="cov", bufs=big_bufs) as cov_pool,
        tc.tile_pool(name="rect", bufs=1) as rect_pool,
        tc.tile_pool(name="grect", bufs=1) as grect_pool,
        tc.tile_pool(name="small", bufs=2) as small_pool,
    ):
        # prefetch every stream's positions up front
        pos_tiles = []
        for s in range(n_streams):
            pos_t = pos_pool.tile([P, sizes[s] * N * D], F32, tag=f"pos{s % 4}")
            nc.sync.dma_start(pos_t[:, :], pos_r[:, offs[s] : offs[s + 1], :])
            pos_tiles.append(pos_t)

        def stream_gen(s):
            par = s % ACTIVE
            Mg = sizes[s]
            posv = pos_tiles[s][:, :].rearrange(
                "p (m i d) -> p m i d", m=Mg, i=N
            )
            A = a_pool.tile([P, Mg * N * N], F32, tag=f"A{par}")
            A4 = A[:, :].rearrange("p (m i j) -> p m i j", m=Mg, i=N)
            Av = A[:, :].rearrange("p (m x) -> p m x", m=Mg)

            # ---- covariance build over the upper rect cover ----
            cov_eng = COV_ENG0 if (s == 0 and COV_ENG0) else COV_ENG
            cov_tail = []
            if COV_FINE and FINE_BANDS is not None:
                # build d^2 over the FINE bands (each with its own column
                # start, so almost no sub-diagonal garbage is computed);
                # sqrt/exp run per band, grouped per rect slot so the ACT
                # table still loads only twice per rect.
                for ri, (r0, r1) in enumerate(RECTS):
                    here = [b for b in FINE_BANDS if r0 <= b[0] < r1]
                    for (b0, b1, beng) in here:
                        bn, bc = b1 - b0, N - b0
                        breg = A4[:, :, b0:b1, b0:]
                        dy = cov_pool.tile(
                            [P, Mg * bn * bc], F32, tag=f"dd{par}"
                        )
                        dyv = dy[:, :].rearrange(
                            "p (m i j) -> p m i j", m=Mg, i=bn
                        )
                        xi = (
                            posv[:, :, b0:b1, 0]
                            .unsqueeze(3)
                            .broadcast_to([P, Mg, bn, bc])
                        )
                        xj = (
                            posv[:, :, b0:, 0]
                            .unsqueeze(2)
                            .broadcast_to([P, Mg, bn, bc])
                        )
                        yi = (
                            posv[:, :, b0:b1, 1]
                            .unsqueeze(3)
                            .broadcast_to([P, Mg, bn, bc])
                        )
                        yj = (
                            posv[:, :, b0:, 1]
                            .unsqueeze(2)
                            .broadcast_to([P, Mg, bn, bc])
                        )
                        if beng == "v":
                            nc.vector.tensor_sub(breg, xi, xj)
                            nc.scalar.square(breg, breg)
                            nc.vector.tensor_sub(dyv, yi, yj)
                            nc.scalar.square(dyv, dyv)
                            nc.vector.tensor_add(breg, breg, dyv)
                        else:
                            _gp_sub(nc, breg, xi, xj)
                            nc.scalar.square(breg, breg)
                            _gp_sub(nc, dyv, yi, yj)
                            nc.scalar.square(dyv, dyv)
                            _gp_add(nc, breg, breg, dyv)
                    for (b0, b1, beng) in here:
                        nc.scalar.sqrt(A4[:, :, b0:b1, b0:], A4[:, :, b0:b1, b0:])
                    for (b0, b1, beng) in here:
                        nc.scalar.activation(
                            A4[:, :, b0:b1, b0:], A4[:, :, b0:b1, b0:],
                            AF.Exp, scale=-PHI,
                        )
                    dseg = Av[
                        :, :, r0 * (N + 1) : (r1 - 1) * (N + 1) + 1 : N + 1
                    ]
                    if TAU_ENG == "g":
                        nc.gpsimd.tensor_scalar_add(dseg, dseg, TAU)
                    else:
                        nc.vector.tensor_scalar_add(dseg, dseg, TAU)
                    yield
            for ri, (r0, r1) in enumerate(
                [] if (COV_FINE and FINE_BANDS is not None) else RECTS
            ):
                nr, ncl = r1 - r0, N - r0
                reg = A4[:, :, r0:r1, r0:]
                sub_a = (
                    (lambda o, a, b: _gp_sub(nc, o, a, b))
                    if cov_eng[ri] == "g"
                    else nc.vector.tensor_sub
                )  # "m": subs on DVE, add on GP
                add_a = (
                    (lambda o, a, b: _gp_add(nc, o, a, b))
                    if cov_eng[ri] in ("g", "m")
                    else nc.vector.tensor_add
                )
                if COV_MERGED:
                    # dd[m,i,j,:] = p[i,:] - p[j,:] in ONE sub (both
                    # coords), square on ACT, strided add -> d^2 in A.
                    pi = (
                        posv[:, :, r0:r1, :]
                        .unsqueeze(3)
                        .broadcast_to([P, Mg, nr, ncl, D])
                    )
                    pj = (
                        posv[:, :, r0:, :]
                        .unsqueeze(2)
                        .broadcast_to([P, Mg, nr, ncl, D])
                    )
                    dd = cov_pool.tile(
                        [P, Mg * nr * ncl * D], F32, tag=f"dd{par}"
                    )
                    ddv = dd[:, :].rearrange(
                        "p (m i j d) -> p m i j d", m=Mg, i=nr, j=ncl
                    )
                    sub_a(ddv, pi, pj)
                    nc.scalar.square(dd[:, :], dd[:, :])
                    add_a(reg, ddv[:, :, :, :, 0], ddv[:, :, :, :, 1])
                else:
                    # dx into A (in-place square), dy in an E-sized tmp;
                    # rows split between GPSIMD (top COV_GP_ROWS[ri]) and
                    # DVE for fine-grained engine balance
                    dy = cov_pool.tile(
                        [P, Mg * nr * ncl], F32, tag=f"dd{par}"
                    )
                    dyv = dy[:, :].rearrange(
                        "p (m i j) -> p m i j", m=Mg, i=nr
                    )
                    csplit = r0 + COV_GP_ROWS[ri]
                    for ceng, pa, pb in (("g", r0, csplit), ("v", csplit, r1)):
                        pn = pb - pa
                        if pn <= 0:
                            continue
                        regp = A4[:, :, pa:pb, r0:]
                        dyp = dyv[:, :, pa - r0 : pb - r0]
                        xi = (
                            posv[:, :, pa:pb, 0]
                            .unsqueeze(3)
                            .broadcast_to([P, Mg, pn, ncl])
                        )
                        xj = (
                            posv[:, :, r0:, 0]
                            .unsqueeze(2)
                            .broadcast_to([P, Mg, pn, ncl])
                        )
                        yi = (
                            posv[:, :, pa:pb, 1]
                            .unsqueeze(3)
                            .broadcast_to([P, Mg, pn, ncl])
                        )
                        yj = (
                            posv[:, :, r0:, 1]
                            .unsqueeze(2)
                            .broadcast_to([P, Mg, pn, ncl])
                        )
                        if ceng == "g":
                            _gp_sub(nc, regp, xi, xj)
                            nc.scalar.square(regp, regp)
                            _gp_sub(nc, dyp, yi, yj)
                            nc.scalar.square(dyp, dyp)
                            _gp_add(nc, regp, regp, dyp)
                        else:
                            nc.vector.tensor_sub(regp, xi, xj)
                            nc.scalar.square(regp, regp)
                            nc.vector.tensor_sub(dyp, yi, yj)
                            nc.scalar.square(dyp, dyp)
                            nc.vector.tensor_add(regp, regp, dyp)
                dseg = Av[:, :, r0 * (N + 1) : (r1 - 1) * (N + 1) + 1 : N + 1]
                if COV_PHASED is True or (COV_PHASED == 2 and ri > 0):
                    cov_tail.append((reg, dseg))
                else:
                    nc.scalar.sqrt(reg, reg)
                    nc.scalar.activation(reg, reg, AF.Exp, scale=-PHI)
                    # nugget on this rect's diagonal segment only: pivot k's
                    # reciprocal then depends on rect band(k) alone, letting
                    # early pivots overlap the remaining rects' cov chains
                    if TAU_ENG == "g":
                        nc.gpsimd.tensor_scalar_add(dseg, dseg, TAU)
                    elif TAU_ENG == "a":
                        nc.scalar.add(dseg, dseg, TAU)
                    else:
                        nc.vector.tensor_scalar_add(dseg, dseg, TAU)
                yield
            if cov_tail:
                for reg, _ in cov_tail:
                    nc.scalar.sqrt(reg, reg)
                for reg, _ in cov_tail:
                    nc.scalar.activation(reg, reg, AF.Exp, scale=-PHI)
                for _, dseg in cov_tail:
                    if TAU_ENG == "g":
                        nc.gpsimd.tensor_scalar_add(dseg, dseg, TAU)
                    elif TAU_ENG == "a":
                        nc.scalar.add(dseg, dseg, TAU)
                    else:
                        nc.vector.tensor_scalar_add(dseg, dseg, TAU)
                yield


            # ---- sweep all 20 pivots (gather-free) ----
            # The raw pivot column/row is read straight out of A as
            # broadcast operands of the rank-1 muls (column k for rows
            # above the pivot, row k for rows below); cr (= c * 1/pivot)
            # is computed from A the same way. All muls are emitted
            # before any sub so the in-place subs (which corrupt the
            # pivot row/col: cr[k] = 1) never race the raw reads.
            # The reciprocal for pivot k+1 is issued right after the sub
            # that finalizes A[k+1,k+1], hiding it under pivot k's tail.
            rK = small_pool.tile([P, Mg], F32, tag=f"r{par}")
            nc.vector.reciprocal(rK[:, :], A4[:, :, 0, 0])
            for k in range(N):
                crK = small_pool.tile([P, Mg * N], F32, tag=f"cr{par}")
                cr3 = crK[:, :].rearrange("p (m i) -> p m i", m=Mg)
                rb = rK[:, :].unsqueeze(2).broadcast_to([P, Mg, N])

                def crmul(which, o, a, b):
                    # "s": col part on GPSIMD; "s2": col part alternates
                    # engines by pivot parity; "r": row part on GPSIMD,
                    # col part on DVE; "r2": row part alternates
                    if which == 0 and (
                        CR_ENG == "s" or (CR_ENG == "s2" and k % 2 == 0)
                    ):
                        _gp_mul(nc, o, a, b)
                    elif which == 1 and (
                        CR_ENG == "r" or (CR_ENG == "r2" and k % 2 == 0)
                    ):
                        _gp_mul(nc, o, a, b)
                    elif CR_ENG == "g":
                        _gp_mul(nc, o, a, b)
                    else:
                        nc.vector.tensor_mul(o, a, b)

                if k:
                    crmul(0, cr3[:, :, :k], A4[:, :, :k, k], rb[:, :, :k])
                crmul(1, cr3[:, :, k:], A4[:, :, k, k:], rb[:, :, k:])

                def c_raw(a, b):
                    """Broadcast AP of raw c[a:b] read from A's storage."""
                    if b <= k + 1:  # rows at or above the pivot: column k
                        return A4[:, :, a:b, k].unsqueeze(3)
                    return A4[:, :, k, a:b].unsqueeze(3)  # below: row k

                last = k == N - 1
                subs = []
                gp_rows = GP_ROWS_EVEN if k % 2 == 0 else GP_ROWS
                if FINE_BANDS is not None:
                    band_iter = []
                    for ri, (r0, r1, eng) in enumerate(FINE_BANDS):
                        if eng == "a":  # alternate by pivot+stream parity
                            eng = "v" if (k + s) % 2 else "g"
                        elif eng == "A":
                            eng = "g" if (k + s) % 2 else "v"
                        if (
                            OWNS_V
                            and eng == "g"
                            and r0 <= k + 1 < r1
                            and k < N - 1
                        ):
                            # dedicated DVE piece for the next-pivot row
                            if k + 2 < r1:
                                band_iter.append(
                                    (ri, r0, r1, "g", k + 2, r1)
                                )
                            if r0 < k + 1:
                                band_iter.append(
                                    (ri, r0, r1, "g", r0, k + 1)
                                )
                            band_iter.append(
                                (100 + ri, r0, r1, "v", k + 1, k + 2)
                            )
                        else:
                            band_iter.append((ri, r0, r1, eng, r0, r1))
                else:
                    band_iter = []
                    for ri, (r0, r1) in enumerate(RECTS):
                        gsplit = r0 + gp_rows[ri]
                        band_iter.append((ri, r0, r1, "g", r0, gsplit))
                        band_iter.append((ri, r0, r1, "v", gsplit, r1))
                for ri, r0, r1, eng, a, b in band_iter:
                    ncl = N - r0
                    if True:
                        if b <= a:
                            continue
                        # rows to update: [a,b) minus the pivot row k
                        # (its results are discarded: the pivot row/col
                        # copies rewrite it below). Rows above the pivot
                        # read c from column k, rows below from row k.
                        if a <= k < b:
                            pieces = [(a, k), (k + 1, b)]
                        else:
                            pieces = [(a, b)]
                        pieces = [(pa, pb) for pa, pb in pieces if pb > pa]
                        if not pieces:
                            continue
                        tot = sum(pb - pa for pa, pb in pieces)
                        pool = grect_pool if eng == "g" else rect_pool
                        tmp = pool.tile(
                            [P, Mg * tot * ncl], F32, tag=f"t{eng}{par}r{ri}"
                        )
                        tv = tmp[:, :].rearrange(
                            "p (m i j) -> p m i j", m=Mg, i=tot
                        )
                        mul = (
                            (lambda o, x, y: _gp_mul(nc, o, x, y))
                            if eng == "g"
                            else nc.vector.tensor_mul
                        )
                        ofs = 0
                        for (pa, pb) in pieces:
                            pn = pb - pa
                            crb = (
                                cr3[:, :, r0:]
                                .unsqueeze(2)
                                .broadcast_to([P, Mg, pn, ncl])
                            )
                            mul(
                                tv[:, :, ofs : ofs + pn],
                                c_raw(pa, pb).broadcast_to([P, Mg, pn, ncl]),
                                crb,
                            )
                            # does this finalize next pivot's diagonal?
                            owns_next = pa <= k + 1 < pb
                            subs.append(
                                (
                                    eng,
                                    A4[:, :, pa:pb, r0:],
                                    tv[:, :, ofs : ofs + pn],
                                    owns_next,
                                )
                            )
                            ofs += pn
                subs.sort(key=lambda t: not t[3])  # next-diag owner first
                for si, (eng, reg, tv, owns_next) in enumerate(subs):
                    if eng == "g":
                        if last:
                            # fold the final negation into the last
                            # pivot: reg <- tv - reg = -(reg - tv)
                            _gp_sub(nc, reg, tv, reg)
                        else:
                            _gp_sub(nc, reg, reg, tv)
                    else:
                        if last:
                            nc.vector.tensor_sub(reg, tv, reg)
                        else:
                            nc.vector.tensor_sub(reg, reg, tv)
                    if owns_next and not last:
                        rK_next = small_pool.tile(
                            [P, Mg], F32, tag=f"r{par}"
                        )
                        nc.vector.reciprocal(
                            rK_next[:, :], A4[:, :, k + 1, k + 1]
                        )
                # pivot row/col (upper parts) <- cr; diag <- -r
                # (the last pivot writes negated values: the whole rect
                # cover holds -result after its reverse subtract)
                if k < N - 1:
                    pcopy = {
                        "v": nc.vector.tensor_copy,
                        "a": nc.scalar.copy,
                        "g": nc.gpsimd.tensor_copy,
                    }[PIVOT_COPY_ENG]
                    if k:
                        pcopy(A4[:, :, :k, k], cr3[:, :, :k])
                    pcopy(A4[:, :, k, k + 1 :], cr3[:, :, k + 1 :])
                    if DIAG_ENG == "a":
                        nc.scalar.mul(A4[:, :, k, k], rK[:, :], -1.0)
                    elif DIAG_ENG == "g":
                        nc.gpsimd.tensor_scalar_mul(
                            A4[:, :, k, k], rK[:, :], -1.0
                        )
                    else:
                        nc.vector.tensor_scalar_mul(
                            A4[:, :, k, k], rK[:, :], -1.0
                        )
                else:
                    if PIVOT_COPY_ENG == "a":
                        nc.scalar.mul(A4[:, :, :k, k], cr3[:, :, :k], -1.0)
                    else:
                        nc.vector.tensor_scalar_mul(
                            A4[:, :, :k, k], cr3[:, :, :k], -1.0
                        )
                    nc.vector.tensor_copy(A4[:, :, k, k], rK[:, :])
                if k < N - 1:
                    rK = rK_next
                yield

            # ---- finalize: mirror upper -> lower (values already negated),
            # in m-halves so the first half's store overlaps the second
            # half's mirror. The last stream has no concurrent work left,
            # so its mirror runs split across the otherwise-idle DVE+GPSIMD.
            tail = s == n_streams - 1
            h = Mg // 2
            for hi, (m0, m1) in enumerate(((0, h), (h, Mg))):
                for i in range(N - 1):
                    if tail:
                        mcopy = (
                            nc.vector.tensor_copy
                            if i % 2
                            else nc.gpsimd.tensor_copy
                        )
                    elif MIRROR_ENG == "a":
                        mcopy = nc.scalar.copy
                    else:
                        mcopy = nc.vector.tensor_copy
                    mcopy(
                        A4[:, m0:m1, i + 1 :, i], A4[:, m0:m1, i, i + 1 :]
                    )
                # the last stream's second half goes out via the ACT
                # HWDGE queue so both halves' transfers overlap (ACT is
                # idle during the pipeline drain; mid-run streams stay on
                # the SP queue where the issue cost is off-engine)
                dma_eng = nc.scalar if (tail and hi == 1) else nc.sync
                dma_eng.dma_start(
                    out_r[:, offs[s] + m0 : offs[s] + m1, :],
                    A[:, m0 * N * N : m1 * N * N],
                )
                yield

        pending = list(range(n_streams))
        active = [stream_gen(pending.pop(0))]
        if COV_PAR_START and pending:
            # run both initial streams' cov builds in parallel, then
            # prime stream 0's sweep so pivots stay phase-offset
            active.append(stream_gen(pending.pop(0)))
            for _ in range(len(RECTS)):
                for gen in active:
                    next(gen)
            for _ in range(STAGGER):
                next(active[0])
        else:
            # prime the first stream so concurrent streams stay offset
            for _ in range(STAGGER):
                next(active[0])
        while pending or active:
            while len(active) < ACTIVE and pending:
                active.append(stream_gen(pending.pop(0)))
            for gen in list(active):
                try:
                    next(gen)
                except StopIteration:
                    active.remove(gen)


_CACHE = {}


def build_nc(b_core=B_CORE, n_streams=None, num_devices=N_CORES):
    if n_streams is None:
        n_streams = N_STREAMS
    key = (b_core, n_streams, num_devices)
    if key in _CACHE:
        return _CACHE[key]
    nc = bacc.Bacc(
        "TRN2", target_bir_lowering=False, debug=False, num_devices=num_devices
    )
    pos_d = nc.dram_tensor("pos", [b_core, N * D], F32, kind="ExternalInput")
    out_d = nc.dram_tensor("out", [b_core, N * N], F32, kind="ExternalOutput")
    with tile.TileContext(nc) as tc:
        emit_kernel(tc, pos_d.ap(), out_d.ap(), b_core, n_streams)
    nc.compile()
    _CACHE[key] = nc
    return nc


def run(pos_full, b_core=B_CORE, n_streams=None, n_cores=N_CORES, **kw):
    """pos_full: [n_cores*b_core, 20, 2] f32 -> [n_cores*b_core, 20, 20] f32."""
    nc = build_nc(b_core, n_streams, n_cores)
    flat = np.ascontiguousarray(
        np.asarray(pos_full, dtype=np.float32).reshape(-1, N * D)
    )
    in_maps = [
        {"pos": flat[i * b_core : (i + 1) * b_core]} for i in range(n_cores)
    ]
    res = run_bass_kernel_spmd(nc, in_maps, core_ids=list(range(n_cores)), **kw)
    out = np.concatenate([r["out"] for r in res.results], axis=0)
    return out.reshape(-1, N, N), res


def kernel(neighbor_positions, edge_list=None):
    out, _ = run(neighbor_positions)
    return out



# revision 32
# speedup vs baseline: 1.1265x; 1.0065x over previous
"""Batched 20x20 SPD covariance-matrix inversion on 8 Trainium2 NeuronCores.

For each of 131072 batches: build C = exp(-1.5 * pairwise_dist(pos)) + 0.01*I
from 20 2-D points, return C^{-1}.

Strategy (per core, data-parallel over batch):
 - batch-major layout: each of 128 SBUF partitions holds Mg matrices' full
   20x20 (400 fp32) in the free dim; N_STREAMS independent streams.
 - symmetric sweep operator (Gauss-Jordan preserving symmetry): only the
   upper triangle is updated each pivot, covered by 4 row-band rectangles.
 - gather-free sweep: the raw pivot column/row is read straight out of A
   as broadcast operands of the rank-1 muls (all muls are emitted before
   any in-place sub so the subs never race those reads), cr = c/pivot is
   computed from A the same way, and the reciprocal of the NEXT pivot is
   issued right after the sub that finalizes its diagonal element. The
   pivot row itself is excluded from the update (its band splits around
   it): those results are discarded by the pivot row/col rewrite anyway.
 - engine split: the rank-1 updates (2 tensor-tensor passes per element)
   are split row-wise between DVE and GPSIMD plain tensor_tensor ops
   (ScalarTensorTensor is BIR-capped at 3D, so unusable here); ACT does
   the cov-build square/sqrt/exp, the pivot row/col writes, and the
   mirror of the upper triangle into the lower.
 - the final negation (sweep yields -A^{-1}) is folded into the last
   pivot's subtracts (reversed operands), so no extra negate pass runs.
 - ACTIVE streams are software-pipelined round-robin at pivot granularity:
   one stream's serial pivot prefix hides under the other's rank-1 work;
   cov builds and finalizes of adjacent streams overlap sweeps the same
   way. All pos DMAs are prefetched at kernel start; outputs are stored
   in m-halves so the first half's DMA overlaps the second's mirror.
"""

import numpy as np

import concourse.bass as bass  # noqa: F401  (registers engine APIs)
import concourse.tile as tile
from concourse import bacc, mybir
from concourse.bass_utils import run_bass_kernel_spmd

N = 20                  # matrix dim
D = 2                   # coord dim
PHI = 1.5
TAU = 0.01
P = 128                 # SBUF partitions
N_CORES = 8
B_TOTAL = 131072
B_CORE = B_TOTAL // N_CORES   # 16384

F32 = mybir.dt.float32
AF = mybir.ActivationFunctionType
OP = mybir.AluOpType

# Upper-triangle rectangle cover: rows [r0,r1) x cols [r0,N)
RECTS = [(0, 5), (5, 10), (10, 15), (15, 20)]

# --- engine-assignment knobs (autotuned via CoreSim) -----------------------
# per rect: how many of its rows (from the top) go to GPSIMD for the rank-1
# update; the rest go to DVE.
GP_ROWS = [5, 1, 1, 5]
# on even pivots one extra row per listed rect goes to GPSIMD (half-row
# granularity for the DVE/GPSIMD balance)
GP_ROWS_EVEN = [5, 1, 1, 5]
# per rect: cov-build tensor-tensor ops (dx, dy, add) engine: "v" DVE, "g" GP
COV_ENG = ["v", "g", "g", "v"]
# per rect: cov rows (from the top) built on GPSIMD; rest on DVE.
# [0,5,5,0] reproduces COV_ENG ["v","g","g","v"].
COV_GP_ROWS = [0, 5, 5, 5]
CR_ENG = "s2"            # cr = c * r:  "v" DVE tensor_mul, "g" GP stt
MIRROR_ENG = "a"        # "a" ACT copy(scale=-1) pre-negate, "v" DVE post
PIVOT_COPY_ENG = "a"    # pivot row/col <- cr copies: "v" DVE, "a" ACT, "g" GP
N_STREAMS = 8           # independent m-slices (Mg = B_CORE/P/N_STREAMS)
# per-stream m sizes (must sum to B_CORE/P); medium ramp-in streams, big
# middle, small tail to shorten the drain once no partner stream remains
STREAM_SIZES = [16, 16, 18, 18, 18, 18, 12, 12]
ACTIVE = 2              # streams pipelined concurrently
STAGGER = 10            # yields to prime stream 0 before starting stream 1
COV_MERGED = False       # cov: one 2-coord sub (needs 2E dd tile) vs dx/dy
DIAG_ENG = "a"          # per-pivot diag<- -r + TAU add: "a" ACT, "v" DVE, "g" GP
COV_PAR_START = False    # run both initial streams' cov builds in parallel
COV_ENG0 = None         # optional cov engine mix for stream 0 (startup ramp)
TAU_ENG = "g"           # per-rect diag nugget add: "v" DVE, "g" GP
# batch all rects' d^2 first, then all sqrts, then all exps: the ACT
# function table holds square+copy in every set but sqrt and exp live in
# different sets, so interleaving sqrt/exp per rect forces a 1283ns
# LoadActFuncSet per switch (~57 loads); phased order pays 2 per stream.
COV_PHASED = False
# Fine-grained rank-1 cover: list of (r0, r1, eng) bands, each updating
# rows [r0,r1) x cols [r0,N).  Narrow bands with their own column start
# carry far less sub-diagonal garbage than the 4x5 rect cover (whose
# lower rows update cols from the parent rect's r0).  GPSIMD tensor ops
# have no per-instruction engine overhead in the cost model, so it takes
# the narrow bottom bands; DVE (60ns init per instr) keeps wide top ones.
# cov-build d^2 over the fine bands (own column starts) instead of the
# coarse rects; sqrt/exp per band grouped per rect slot.
COV_FINE = False
# Carve row k+1 out of a GPSIMD band into a dedicated DVE piece so the
# diag-finalizing sub and the next pivot's reciprocal stay on one engine
# (no cross-engine semaphore on the serial pivot chain).
OWNS_V = False
# None falls back to the baseline RECTS/GP_ROWS path.
FINE_BANDS = [
    (0, 1, "g"), (1, 2, "g"), (2, 3, "g"), (3, 4, "g"),
    (4, 7, "v"), (7, 10, "v"),
    (10, 11, "g"), (11, 12, "g"), (12, 13, "g"), (13, 14, "g"),
    (14, 15, "g"), (15, 16, "g"), (16, 17, "g"), (17, 18, "g"),
    (18, 19, "g"), (19, 20, "g"),
]


def _gp_mul(nc, out, a, b):
    """out = a * b on GPSIMD (plain TensorTensor: >=4D APs are BIR-legal,
    unlike ScalarTensorTensor which the BIR verifier caps at 3D)."""
    nc.gpsimd.tensor_tensor(out, a, b, OP.mult)


def _gp_sub(nc, out, a, b):
    """out = a - b on GPSIMD."""
    nc.gpsimd.tensor_tensor(out, a, b, OP.subtract)


def _gp_add(nc, out, a, b):
    nc.gpsimd.tensor_tensor(out, a, b, OP.add)


def emit_kernel(tc, pos_ap, out_ap, b_core, n_streams):
    """Emit the per-core program. pos: [b_core, 40] f32, out: [b_core, 400] f32."""
    nc = tc.nc
    m_total = b_core // P
    if STREAM_SIZES is not None:
        sizes = list(STREAM_SIZES)
        n_streams = len(sizes)
    else:
        sizes = [m_total // n_streams] * n_streams
    assert sum(sizes) == m_total
    offs = [0]
    for sz in sizes:
        offs.append(offs[-1] + sz)

    pos_r = pos_ap.rearrange("(p m) f -> p m f", p=P)
    out_r = out_ap.rearrange("(p m) f -> p m f", p=P)
    big_bufs = 2 if (ACTIVE <= 2 and max(sizes) < 32) else 1

    with (
        tc.tile_pool(name="pos", bufs=1) as pos_pool,
        tc.tile_pool(name="A", bufs=big_bufs) as a_pool,
        tc.tile_pool(name="cov", bufs=big_bufs) as cov_pool,
        tc.tile_pool(name="rect", bufs=1) as rect_pool,
        tc.tile_pool(name="grect", bufs=1) as grect_pool,
        tc.tile_pool(name="small", bufs=2) as small_pool,
    ):
        # prefetch every stream's positions up front
        pos_tiles = []
        for s in range(n_streams):
            pos_t = pos_pool.tile([P, sizes[s] * N * D], F32, tag=f"pos{s % 4}")
            nc.sync.dma_start(pos_t[:, :], pos_r[:, offs[s] : offs[s + 1], :])
            pos_tiles.append(pos_t)

        def stream_gen(s):
            par = s % ACTIVE
            Mg = sizes[s]
            posv = pos_tiles[s][:, :].rearrange(
                "p (m i d) -> p m i d", m=Mg, i=N
            )
            A = a_pool.tile([P, Mg * N * N], F32, tag=f"A{par}")
            A4 = A[:, :].rearrange("p (m i j) -> p m i j", m=Mg, i=N)
            Av = A[:, :].rearrange("p (m x) -> p m x", m=Mg)

            # ---- covariance build over the upper rect cover ----
            cov_eng = COV_ENG0 if (s == 0 and COV_ENG0) else COV_ENG
            cov_tail = []
            if COV_FINE and FINE_BANDS is not None:
                # build d^2 over the FINE bands (each with its own column
                # start, so almost no sub-diagonal garbage is computed);
                # sqrt/exp run per band, grouped per rect slot so the ACT
                # table still loads only twice per rect.
                for ri, (r0, r1) in enumerate(RECTS):
                    here = [b for b in FINE_BANDS if r0 <= b[0] < r1]
                    for (b0, b1, beng) in here:
                        bn, bc = b1 - b0, N - b0
                        breg = A4[:, :, b0:b1, b0:]
                        dy = cov_pool.tile(
                            [P, Mg * bn * bc], F32, tag=f"dd{par}"
                        )
                        dyv = dy[:, :].rearrange(
                            "p (m i j) -> p m i j", m=Mg, i=bn
                        )
                        xi = (
                            posv[:, :, b0:b1, 0]
                            .unsqueeze(3)
                            .broadcast_to([P, Mg, bn, bc])
                        )
                        xj = (
                            posv[:, :, b0:, 0]
                            .unsqueeze(2)
                            .broadcast_to([P, Mg, bn, bc])
                        )
                        yi = (
                            posv[:, :, b0:b1, 1]
                            .unsqueeze(3)
                            .broadcast_to([P, Mg, bn, bc])
                        )
                        yj = (
                            posv[:, :, b0:, 1]
                            .unsqueeze(2)
                            .broadcast_to([P, Mg, bn, bc])
                        )
                        if beng == "v":
                            nc.vector.tensor_sub(breg, xi, xj)
                            nc.scalar.square(breg, breg)
                            nc.vector.tensor_sub(dyv, yi, yj)
                            nc.scalar.square(dyv, dyv)
                            nc.vector.tensor_add(breg, breg, dyv)
                        else:
                            _gp_sub(nc, breg, xi, xj)
                            nc.scalar.square(breg, breg)
                            _gp_sub(nc, dyv, yi, yj)
                            nc.scalar.square(dyv, dyv)
                            _gp_add(nc, breg, breg, dyv)
                    for (b0, b1, beng) in here:
                        nc.scalar.sqrt(A4[:, :, b0:b1, b0:], A4[:, :, b0:b1, b0:])
                    for (b0, b1, beng) in here:
                        nc.scalar.activation(
                            A4[:, :, b0:b1, b0:], A4[:, :, b0:b1, b0:],
                            AF.Exp, scale=-PHI,
                        )
                    dseg = Av[
                        :, :, r0 * (N + 1) : (r1 - 1) * (N + 1) + 1 : N + 1
                    ]
                    if TAU_ENG == "g":
                        nc.gpsimd.tensor_scalar_add(dseg, dseg, TAU)
                    else:
                        nc.vector.tensor_scalar_add(dseg, dseg, TAU)
                    yield
            for ri, (r0, r1) in enumerate(
                [] if (COV_FINE and FINE_BANDS is not None) else RECTS
            ):
                nr, ncl = r1 - r0, N - r0
                reg = A4[:, :, r0:r1, r0:]
                sub_a = (
                    (lambda o, a, b: _gp_sub(nc, o, a, b))
                    if cov_eng[ri] == "g"
                    else nc.vector.tensor_sub
                )  # "m": subs on DVE, add on GP
                add_a = (
                    (lambda o, a, b: _gp_add(nc, o, a, b))
                    if cov_eng[ri] in ("g", "m")
                    else nc.vector.tensor_add
                )
                if COV_MERGED:
                    # dd[m,i,j,:] = p[i,:] - p[j,:] in ONE sub (both
                    # coords), square on ACT, strided add -> d^2 in A.
                    pi = (
                        posv[:, :, r0:r1, :]
                        .unsqueeze(3)
                        .broadcast_to([P, Mg, nr, ncl, D])
                    )
                    pj = (
                        posv[:, :, r0:, :]
                        .unsqueeze(2)
                        .broadcast_to([P, Mg, nr, ncl, D])
                    )
                    dd = cov_pool.tile(
                        [P, Mg * nr * ncl * D], F32, tag=f"dd{par}"
                    )
                    ddv = dd[:, :].rearrange(
                        "p (m i j d) -> p m i j d", m=Mg, i=nr, j=ncl
                    )
                    sub_a(ddv, pi, pj)
                    nc.scalar.square(dd[:, :], dd[:, :])
                    add_a(reg, ddv[:, :, :, :, 0], ddv[:, :, :, :, 1])
                else:
                    # dx into A (in-place square), dy in an E-sized tmp;
                    # rows split between GPSIMD (top COV_GP_ROWS[ri]) and
                    # DVE for fine-grained engine balance
                    dy = cov_pool.tile(
                        [P, Mg * nr * ncl], F32, tag=f"dd{par}"
                    )
                    dyv = dy[:, :].rearrange(
                        "p (m i j) -> p m i j", m=Mg, i=nr
                    )
                    csplit = r0 + COV_GP_ROWS[ri]
                    for ceng, pa, pb in (("g", r0, csplit), ("v", csplit, r1)):
                        pn = pb - pa
                        if pn <= 0:
                            continue
                        regp = A4[:, :, pa:pb, r0:]
                        dyp = dyv[:, :, pa - r0 : pb - r0]
                        xi = (
                            posv[:, :, pa:pb, 0]
                            .unsqueeze(3)
                            .broadcast_to([P, Mg, pn, ncl])
                        )
                        xj = (
                            posv[:, :, r0:, 0]
                            .unsqueeze(2)
                            .broadcast_to([P, Mg, pn, ncl])
                        )
                        yi = (
                            posv[:, :, pa:pb, 1]
                            .unsqueeze(3)
                            .broadcast_to([P, Mg, pn, ncl])
                        )
                        yj = (
                            posv[:, :, r0:, 1]
                            .unsqueeze(2)
                            .broadcast_to([P, Mg, pn, ncl])
                        )
                        if ceng == "g":
                            _gp_sub(nc, regp, xi, xj)
                            nc.scalar.square(regp, regp)
                            _gp_sub(nc, dyp, yi, yj)
                            nc.scalar.square(dyp, dyp)
                            _gp_add(nc, regp, regp, dyp)
                        else:
                            nc.vector.tensor_sub(regp, xi, xj)
                            nc.scalar.square(regp, regp)
                            nc.vector.tensor_sub(dyp, yi, yj)
                            nc.scalar.square(dyp, dyp)
                            nc.vector.tensor_add(regp, regp, dyp)
                dseg = Av[:, :, r0 * (N + 1) : (r1 - 1) * (N + 1) + 1 : N + 1]
                if COV_PHASED is True or (COV_PHASED == 2 and ri > 0):
                    cov_tail.append((reg, dseg))
                else:
                    nc.scalar.sqrt(reg, reg)
                    nc.scalar.activation(reg, reg, AF.Exp, scale=-PHI)
                    # nugget on this rect's diagonal segment only: pivot k's
                    # reciprocal then depends on rect band(k) alone, letting
                    # early pivots overlap the remaining rects' cov chains
                    if TAU_ENG == "g":
                        nc.gpsimd.tensor_scalar_add(dseg, dseg, TAU)
                    elif TAU_ENG == "a":
                        nc.scalar.add(dseg, dseg, TAU)
                    else:
                        nc.vector.tensor_scalar_add(dseg, dseg, TAU)
                yield
            if cov_tail:
                for reg, _ in cov_tail:
                    nc.scalar.sqrt(reg, reg)
                for reg, _ in cov_tail:
                    nc.scalar.activation(reg, reg, AF.Exp, scale=-PHI)
                for _, dseg in cov_tail:
                    if TAU_ENG == "g":
                        nc.gpsimd.tensor_scalar_add(dseg, dseg, TAU)
                    elif TAU_ENG == "a":
                        nc.scalar.add(dseg, dseg, TAU)
                    else:
                        nc.vector.tensor_scalar_add(dseg, dseg, TAU)
                yield


            # ---- sweep all 20 pivots (gather-free) ----
            # The raw pivot column/row is read straight out of A as
            # broadcast operands of the rank-1 muls (column k for rows
            # above the pivot, row k for rows below); cr (= c * 1/pivot)
            # is computed from A the same way. All muls are emitted
            # before any sub so the in-place subs (which corrupt the
            # pivot row/col: cr[k] = 1) never race the raw reads.
            # The reciprocal for pivot k+1 is issued right after the sub
            # that finalizes A[k+1,k+1], hiding it under pivot k's tail.
            rK = small_pool.tile([P, Mg], F32, tag=f"r{par}")
            nc.vector.reciprocal(rK[:, :], A4[:, :, 0, 0])
            for k in range(N):
                crK = small_pool.tile([P, Mg * N], F32, tag=f"cr{par}")
                cr3 = crK[:, :].rearrange("p (m i) -> p m i", m=Mg)
                rb = rK[:, :].unsqueeze(2).broadcast_to([P, Mg, N])

                def crmul(which, o, a, b):
                    # "s": col part on GPSIMD; "s2": col part alternates
                    # engines by pivot parity; "r": row part on GPSIMD,
                    # col part on DVE; "r2": row part alternates
                    if which == 0 and (
                        CR_ENG == "s" or (CR_ENG == "s2" and k % 2 == 0)
                    ):
                        _gp_mul(nc, o, a, b)
                    elif which == 1 and (
                        CR_ENG == "r" or (CR_ENG == "r2" and k % 2 == 0)
                    ):
                        _gp_mul(nc, o, a, b)
                    elif CR_ENG == "g":
                        _gp_mul(nc, o, a, b)
                    else:
                        nc.vector.tensor_mul(o, a, b)

                if k:
                    crmul(0, cr3[:, :, :k], A4[:, :, :k, k], rb[:, :, :k])
                crmul(1, cr3[:, :, k:], A4[:, :, k, k:], rb[:, :, k:])

                def c_raw(a, b):
                    """Broadcast AP of raw c[a:b] read from A's storage."""
                    if b <= k + 1:  # rows at or above the pivot: column k
                        return A4[:, :, a:b, k].unsqueeze(3)
                    return A4[:, :, k, a:b].unsqueeze(3)  # below: row k

                last = k == N - 1
                subs = []
                gp_rows = GP_ROWS_EVEN if k % 2 == 0 else GP_ROWS
                if FINE_BANDS is not None:
                    band_iter = []
                    for ri, (r0, r1, eng) in enumerate(FINE_BANDS):
                        if eng == "a":  # alternate by pivot+stream parity
                            eng = "v" if (k + s) % 2 else "g"
                        elif eng == "A":
                            eng = "g" if (k + s) % 2 else "v"
                        if (
                            OWNS_V
                            and eng == "g"
                            and r0 <= k + 1 < r1
                            and k < N - 1
                        ):
                            # dedicated DVE piece for the next-pivot row
                            if k + 2 < r1:
                                band_iter.append(
                                    (ri, r0, r1, "g", k + 2, r1)
                                )
                            if r0 < k + 1:
                                band_iter.append(
                                    (ri, r0, r1, "g", r0, k + 1)
                                )
                            band_iter.append(
                                (100 + ri, r0, r1, "v", k + 1, k + 2)
                            )
                        else:
                            band_iter.append((ri, r0, r1, eng, r0, r1))
                else:
                    band_iter = []
                    for ri, (r0, r1) in enumerate(RECTS):
                        gsplit = r0 + gp_rows[ri]
                        band_iter.append((ri, r0, r1, "g", r0, gsplit))
                        band_iter.append((ri, r0, r1, "v", gsplit, r1))
                for ri, r0, r1, eng, a, b in band_iter:
                    ncl = N - r0
                    if True:
                        if b <= a:
                            continue
                        # rows to update: [a,b) minus the pivot row k
                        # (its results are discarded: the pivot row/col
                        # copies rewrite it below). Rows above the pivot
                        # read c from column k, rows below from row k.
                        if a <= k < b:
                            pieces = [(a, k), (k + 1, b)]
                        else:
                            pieces = [(a, b)]
                        pieces = [(pa, pb) for pa, pb in pieces if pb > pa]
                        if not pieces:
                            continue
                        tot = sum(pb - pa for pa, pb in pieces)
                        pool = grect_pool if eng == "g" else rect_pool
                        tmp = pool.tile(
                            [P, Mg * tot * ncl], F32, tag=f"t{eng}{par}r{ri}"
                        )
                        tv = tmp[:, :].rearrange(
                            "p (m i j) -> p m i j", m=Mg, i=tot
                        )
                        mul = (
                            (lambda o, x, y: _gp_mul(nc, o, x, y))
                            if eng == "g"
                            else nc.vector.tensor_mul
                        )
                        ofs = 0
                        for (pa, pb) in pieces:
                            pn = pb - pa
                            crb = (
                                cr3[:, :, r0:]
                                .unsqueeze(2)
                                .broadcast_to([P, Mg, pn, ncl])
                            )
                            mul(
                                tv[:, :, ofs : ofs + pn],
                                c_raw(pa, pb).broadcast_to([P, Mg, pn, ncl]),
                                crb,
                            )
                            # does this finalize next pivot's diagonal?
                            owns_next = pa <= k + 1 < pb
                            subs.append(
                                (
                                    eng,
                                    A4[:, :, pa:pb, r0:],
                                    tv[:, :, ofs : ofs + pn],
                                    owns_next,
                                )
                            )
                            ofs += pn
                subs.sort(key=lambda t: not t[3])  # next-diag owner first
                for si, (eng, reg, tv, owns_next) in enumerate(subs):
                    if eng == "g":
                        if last:
                            # fold the final negation into the last
                            # pivot: reg <- tv - reg = -(reg - tv)
                            _gp_sub(nc, reg, tv, reg)
                        else:
                            _gp_sub(nc, reg, reg, tv)
                    else:
                        if last:
                            nc.vector.tensor_sub(reg, tv, reg)
                        else:
                            nc.vector.tensor_sub(reg, reg, tv)
                    if owns_next and not last:
                        rK_next = small_pool.tile(
                            [P, Mg], F32, tag=f"r{par}"
                        )
                        nc.vector.reciprocal(
                            rK_next[:, :], A4[:, :, k + 1, k + 1]
                        )
                # pivot row/col (upper parts) <- cr; diag <- -r
                # (the last pivot writes negated values: the whole rect
                # cover holds -result after its reverse subtract)
                if k < N - 1:
                    pcopy = {
                        "v": nc.vector.tensor_copy,
                        "a": nc.scalar.copy,
                        "g": nc.gpsimd.tensor_copy,
                    }[PIVOT_COPY_ENG]
                    if k:
                        pcopy(A4[:, :, :k, k], cr3[:, :, :k])
                    pcopy(A4[:, :, k, k + 1 :], cr3[:, :, k + 1 :])
                    if DIAG_ENG == "a":
                        nc.scalar.mul(A4[:, :, k, k], rK[:, :], -1.0)
                    elif DIAG_ENG == "g":
                        nc.gpsimd.tensor_scalar_mul(
                            A4[:, :, k, k], rK[:, :], -1.0
                        )
                    else:
                        nc.vector.tensor_scalar_mul(
                            A4[:, :, k, k], rK[:, :], -1.0
                        )
                else:
                    if PIVOT_COPY_ENG == "a":
                        nc.scalar.mul(A4[:, :, :k, k], cr3[:, :, :k], -1.0)
                    else:
                        nc.vector.tensor_scalar_mul(
                            A4[:, :, :k, k], cr3[:, :, :k], -1.0
                        )
                    nc.vector.tensor_copy(A4[:, :, k, k], rK[:, :])
                if k < N - 1:
                    rK = rK_next
                yield

            # ---- finalize: mirror upper -> lower (values already negated),
            # in m-halves so the first half's store overlaps the second
            # half's mirror. The last stream has no concurrent work left,
            # so its mirror runs split across the otherwise-idle DVE+GPSIMD.
            tail = s == n_streams - 1
            h = Mg // 2
            for hi, (m0, m1) in enumerate(((0, h), (h, Mg))):
                for i in range(N - 1):
                    if tail:
                        mcopy = (
                            nc.vector.tensor_copy
                            if i % 2
                            else nc.gpsimd.tensor_copy
                        )
                    elif MIRROR_ENG == "a":
                        mcopy = nc.scalar.copy
                    else:
                        mcopy = nc.vector.tensor_copy
                    mcopy(
                        A4[:, m0:m1, i + 1 :, i], A4[:, m0:m1, i, i + 1 :]
                    )
                # the last stream's second half goes out via the ACT
                # HWDGE queue so both halves' transfers overlap (ACT is
                # idle during the pipeline drain; mid-run streams stay on
                # the SP queue where the issue cost is off-engine)
                dma_eng = nc.scalar if (tail and hi == 1) else nc.sync
                dma_eng.dma_start(
                    out_r[:, offs[s] + m0 : offs[s] + m1, :],
                    A[:, m0 * N * N : m1 * N * N],
                )
                yield

        pending = list(range(n_streams))
        active = [stream_gen(pending.pop(0))]
        if COV_PAR_START and pending:
            # run both initial streams' cov builds in parallel, then
            # prime stream 0's sweep so pivots stay phase-offset
            active.append(stream_gen(pending.pop(0)))
            for _ in range(len(RECTS)):
                for gen in active:
                    next(gen)
            for _ in range(STAGGER):
                next(active[0])
        else:
            # prime the first stream so concurrent streams stay offset
            for _ in range(STAGGER):
                next(active[0])
        while pending or active:
            while len(active) < ACTIVE and pending:
                active.append(stream_gen(pending.pop(0)))
            for gen in list(active):
                try:
                    next(gen)
                except StopIteration:
                    active.remove(gen)


_CACHE = {}


def build_nc(b_core=B_CORE, n_streams=None, num_devices=N_CORES):
    if n_streams is None:
        n_streams = N_STREAMS
    key = (b_core, n_streams, num_devices)
    if key in _CACHE:
        return _CACHE[key]
    nc = bacc.Bacc(
        "TRN2", target_bir_lowering=False, debug=False, num_devices=num_devices
    )
    pos_d = nc.dram_tensor("pos", [b_core, N * D], F32, kind="ExternalInput")
    out_d = nc.dram_tensor("out", [b_core, N * N], F32, kind="ExternalOutput")
    with tile.TileContext(nc) as tc:
        emit_kernel(tc, pos_d.ap(), out_d.ap(), b_core, n_streams)
    nc.compile()
    _CACHE[key] = nc
    return nc


def run(pos_full, b_core=B_CORE, n_streams=None, n_cores=N_CORES, **kw):
    """pos_full: [n_cores*b_core, 20, 2] f32 -> [n_cores*b_core, 20, 20] f32."""
    nc = build_nc(b_core, n_streams, n_cores)
    flat = np.ascontiguousarray(
        np.asarray(pos_full, dtype=np.float32).reshape(-1, N * D)
    )
    in_maps = [
        {"pos": flat[i * b_core : (i + 1) * b_core]} for i in range(n_cores)
    ]
    res = run_bass_kernel_spmd(nc, in_maps, core_ids=list(range(n_cores)), **kw)
    out = np.concatenate([r["out"] for r in res.results], axis=0)
    return out.reshape(-1, N, N), res


def kernel(neighbor_positions, edge_list=None):
    out, _ = run(neighbor_positions)
    return out



# revision 33
# speedup vs baseline: 1.1269x; 1.0003x over previous
"""Batched 20x20 SPD covariance-matrix inversion on 8 Trainium2 NeuronCores.

For each of 131072 batches: build C = exp(-1.5 * pairwise_dist(pos)) + 0.01*I
from 20 2-D points, return C^{-1}.

Strategy (per core, data-parallel over batch):
 - batch-major layout: each of 128 SBUF partitions holds Mg matrices' full
   20x20 (400 fp32) in the free dim; N_STREAMS independent streams.
 - symmetric sweep operator (Gauss-Jordan preserving symmetry): only the
   upper triangle is updated each pivot, covered by 4 row-band rectangles.
 - gather-free sweep: the raw pivot column/row is read straight out of A
   as broadcast operands of the rank-1 muls (all muls are emitted before
   any in-place sub so the subs never race those reads), cr = c/pivot is
   computed from A the same way, and the reciprocal of the NEXT pivot is
   issued right after the sub that finalizes its diagonal element. The
   pivot row itself is excluded from the update (its band splits around
   it): those results are discarded by the pivot row/col rewrite anyway.
 - engine split: the rank-1 updates (2 tensor-tensor passes per element)
   are split row-wise between DVE and GPSIMD plain tensor_tensor ops
   (ScalarTensorTensor is BIR-capped at 3D, so unusable here); ACT does
   the cov-build square/sqrt/exp, the pivot row/col writes, and the
   mirror of the upper triangle into the lower.
 - the final negation (sweep yields -A^{-1}) is folded into the last
   pivot's subtracts (reversed operands), so no extra negate pass runs.
 - ACTIVE streams are software-pipelined round-robin at pivot granularity:
   one stream's serial pivot prefix hides under the other's rank-1 work;
   cov builds and finalizes of adjacent streams overlap sweeps the same
   way. All pos DMAs are prefetched at kernel start; outputs are stored
   in m-halves so the first half's DMA overlaps the second's mirror.
"""

import numpy as np

import concourse.bass as bass  # noqa: F401  (registers engine APIs)
import concourse.tile as tile
from concourse import bacc, mybir
from concourse.bass_utils import run_bass_kernel_spmd

N = 20                  # matrix dim
D = 2                   # coord dim
PHI = 1.5
TAU = 0.01
P = 128                 # SBUF partitions
N_CORES = 8
B_TOTAL = 131072
B_CORE = B_TOTAL // N_CORES   # 16384

F32 = mybir.dt.float32
AF = mybir.ActivationFunctionType
OP = mybir.AluOpType

# Upper-triangle rectangle cover: rows [r0,r1) x cols [r0,N)
RECTS = [(0, 5), (5, 10), (10, 15), (15, 20)]

# --- engine-assignment knobs (autotuned via CoreSim) -----------------------
# per rect: how many of its rows (from the top) go to GPSIMD for the rank-1
# update; the rest go to DVE.
GP_ROWS = [5, 1, 1, 5]
# on even pivots one extra row per listed rect goes to GPSIMD (half-row
# granularity for the DVE/GPSIMD balance)
GP_ROWS_EVEN = [5, 1, 1, 5]
# per rect: cov-build tensor-tensor ops (dx, dy, add) engine: "v" DVE, "g" GP
COV_ENG = ["v", "g", "g", "v"]
# per rect: cov rows (from the top) built on GPSIMD; rest on DVE.
# [0,5,5,0] reproduces COV_ENG ["v","g","g","v"].
COV_GP_ROWS = [0, 5, 5, 5]
CR_ENG = "s2"            # cr = c * r:  "v" DVE tensor_mul, "g" GP stt
MIRROR_ENG = "a"        # "a" ACT copy(scale=-1) pre-negate, "v" DVE post
PIVOT_COPY_ENG = "a"    # pivot row/col <- cr copies: "v" DVE, "a" ACT, "g" GP
N_STREAMS = 8           # independent m-slices (Mg = B_CORE/P/N_STREAMS)
# per-stream m sizes (must sum to B_CORE/P); medium ramp-in streams, big
# middle, small tail to shorten the drain once no partner stream remains
STREAM_SIZES = [16, 18, 18, 18, 18, 16, 12, 12]
ACTIVE = 2              # streams pipelined concurrently
STAGGER = 10            # yields to prime stream 0 before starting stream 1
COV_MERGED = False       # cov: one 2-coord sub (needs 2E dd tile) vs dx/dy
DIAG_ENG = "a"          # per-pivot diag<- -r + TAU add: "a" ACT, "v" DVE, "g" GP
COV_PAR_START = False    # run both initial streams' cov builds in parallel
COV_ENG0 = None         # optional cov engine mix for stream 0 (startup ramp)
TAU_ENG = "g"           # per-rect diag nugget add: "v" DVE, "g" GP
# batch all rects' d^2 first, then all sqrts, then all exps: the ACT
# function table holds square+copy in every set but sqrt and exp live in
# different sets, so interleaving sqrt/exp per rect forces a 1283ns
# LoadActFuncSet per switch (~57 loads); phased order pays 2 per stream.
COV_PHASED = False
# Fine-grained rank-1 cover: list of (r0, r1, eng) bands, each updating
# rows [r0,r1) x cols [r0,N).  Narrow bands with their own column start
# carry far less sub-diagonal garbage than the 4x5 rect cover (whose
# lower rows update cols from the parent rect's r0).  GPSIMD tensor ops
# have no per-instruction engine overhead in the cost model, so it takes
# the narrow bottom bands; DVE (60ns init per instr) keeps wide top ones.
# cov-build d^2 over the fine bands (own column starts) instead of the
# coarse rects; sqrt/exp per band grouped per rect slot.
COV_FINE = False
# Carve row k+1 out of a GPSIMD band into a dedicated DVE piece so the
# diag-finalizing sub and the next pivot's reciprocal stay on one engine
# (no cross-engine semaphore on the serial pivot chain).
OWNS_V = False
# None falls back to the baseline RECTS/GP_ROWS path.
FINE_BANDS = [
    (0, 1, "g"), (1, 2, "g"), (2, 3, "g"), (3, 4, "g"),
    (4, 7, "v"), (7, 10, "v"),
    (10, 11, "g"), (11, 12, "g"), (12, 13, "g"), (13, 14, "g"),
    (14, 15, "g"), (15, 16, "g"), (16, 17, "g"), (17, 18, "g"),
    (18, 19, "g"), (19, 20, "g"),
]


def _gp_mul(nc, out, a, b):
    """out = a * b on GPSIMD (plain TensorTensor: >=4D APs are BIR-legal,
    unlike ScalarTensorTensor which the BIR verifier caps at 3D)."""
    nc.gpsimd.tensor_tensor(out, a, b, OP.mult)


def _gp_sub(nc, out, a, b):
    """out = a - b on GPSIMD."""
    nc.gpsimd.tensor_tensor(out, a, b, OP.subtract)


def _gp_add(nc, out, a, b):
    nc.gpsimd.tensor_tensor(out, a, b, OP.add)


def emit_kernel(tc, pos_ap, out_ap, b_core, n_streams):
    """Emit the per-core program. pos: [b_core, 40] f32, out: [b_core, 400] f32."""
    nc = tc.nc
    m_total = b_core // P
    if STREAM_SIZES is not None:
        sizes = list(STREAM_SIZES)
        n_streams = len(sizes)
    else:
        sizes = [m_total // n_streams] * n_streams
    assert sum(sizes) == m_total
    offs = [0]
    for sz in sizes:
        offs.append(offs[-1] + sz)

    pos_r = pos_ap.rearrange("(p m) f -> p m f", p=P)
    out_r = out_ap.rearrange("(p m) f -> p m f", p=P)
    big_bufs = 2 if (ACTIVE <= 2 and max(sizes) < 32) else 1

    with (
        tc.tile_pool(name="pos", bufs=1) as pos_pool,
        tc.tile_pool(name="A", bufs=big_bufs) as a_pool,
        tc.tile_pool(name="cov", bufs=big_bufs) as cov_pool,
        tc.tile_pool(name="rect", bufs=1) as rect_pool,
        tc.tile_pool(name="grect", bufs=1) as grect_pool,
        tc.tile_pool(name="small", bufs=2) as small_pool,
    ):
        # prefetch every stream's positions up front
        pos_tiles = []
        for s in range(n_streams):
            pos_t = pos_pool.tile([P, sizes[s] * N * D], F32, tag=f"pos{s % 4}")
            nc.sync.dma_start(pos_t[:, :], pos_r[:, offs[s] : offs[s + 1], :])
            pos_tiles.append(pos_t)

        def stream_gen(s):
            par = s % ACTIVE
            Mg = sizes[s]
            posv = pos_tiles[s][:, :].rearrange(
                "p (m i d) -> p m i d", m=Mg, i=N
            )
            A = a_pool.tile([P, Mg * N * N], F32, tag=f"A{par}")
            A4 = A[:, :].rearrange("p (m i j) -> p m i j", m=Mg, i=N)
            Av = A[:, :].rearrange("p (m x) -> p m x", m=Mg)

            # ---- covariance build over the upper rect cover ----
            cov_eng = COV_ENG0 if (s == 0 and COV_ENG0) else COV_ENG
            cov_tail = []
            if COV_FINE and FINE_BANDS is not None:
                # build d^2 over the FINE bands (each with its own column
                # start, so almost no sub-diagonal garbage is computed);
                # sqrt/exp run per band, grouped per rect slot so the ACT
                # table still loads only twice per rect.
                for ri, (r0, r1) in enumerate(RECTS):
                    here = [b for b in FINE_BANDS if r0 <= b[0] < r1]
                    for (b0, b1, beng) in here:
                        bn, bc = b1 - b0, N - b0
                        breg = A4[:, :, b0:b1, b0:]
                        dy = cov_pool.tile(
                            [P, Mg * bn * bc], F32, tag=f"dd{par}"
                        )
                        dyv = dy[:, :].rearrange(
                            "p (m i j) -> p m i j", m=Mg, i=bn
                        )
                        xi = (
                            posv[:, :, b0:b1, 0]
                            .unsqueeze(3)
                            .broadcast_to([P, Mg, bn, bc])
                        )
                        xj = (
                            posv[:, :, b0:, 0]
                            .unsqueeze(2)
                            .broadcast_to([P, Mg, bn, bc])
                        )
                        yi = (
                            posv[:, :, b0:b1, 1]
                            .unsqueeze(3)
                            .broadcast_to([P, Mg, bn, bc])
                        )
                        yj = (
                            posv[:, :, b0:, 1]
                            .unsqueeze(2)
                            .broadcast_to([P, Mg, bn, bc])
                        )
                        if beng == "v":
                            nc.vector.tensor_sub(breg, xi, xj)
                            nc.scalar.square(breg, breg)
                            nc.vector.tensor_sub(dyv, yi, yj)
                            nc.scalar.square(dyv, dyv)
                            nc.vector.tensor_add(breg, breg, dyv)
                        else:
                            _gp_sub(nc, breg, xi, xj)
                            nc.scalar.square(breg, breg)
                            _gp_sub(nc, dyv, yi, yj)
                            nc.scalar.square(dyv, dyv)
                            _gp_add(nc, breg, breg, dyv)
                    for (b0, b1, beng) in here:
                        nc.scalar.sqrt(A4[:, :, b0:b1, b0:], A4[:, :, b0:b1, b0:])
                    for (b0, b1, beng) in here:
                        nc.scalar.activation(
                            A4[:, :, b0:b1, b0:], A4[:, :, b0:b1, b0:],
                            AF.Exp, scale=-PHI,
                        )
                    dseg = Av[
                        :, :, r0 * (N + 1) : (r1 - 1) * (N + 1) + 1 : N + 1
                    ]
                    if TAU_ENG == "g":
                        nc.gpsimd.tensor_scalar_add(dseg, dseg, TAU)
                    else:
                        nc.vector.tensor_scalar_add(dseg, dseg, TAU)
                    yield
            for ri, (r0, r1) in enumerate(
                [] if (COV_FINE and FINE_BANDS is not None) else RECTS
            ):
                nr, ncl = r1 - r0, N - r0
                reg = A4[:, :, r0:r1, r0:]
                sub_a = (
                    (lambda o, a, b: _gp_sub(nc, o, a, b))
                    if cov_eng[ri] == "g"
                    else nc.vector.tensor_sub
                )  # "m": subs on DVE, add on GP
                add_a = (
                    (lambda o, a, b: _gp_add(nc, o, a, b))
                    if cov_eng[ri] in ("g", "m")
                    else nc.vector.tensor_add
                )
                if COV_MERGED:
                    # dd[m,i,j,:] = p[i,:] - p[j,:] in ONE sub (both
                    # coords), square on ACT, strided add -> d^2 in A.
                    pi = (
                        posv[:, :, r0:r1, :]
                        .unsqueeze(3)
                        .broadcast_to([P, Mg, nr, ncl, D])
                    )
                    pj = (
                        posv[:, :, r0:, :]
                        .unsqueeze(2)
                        .broadcast_to([P, Mg, nr, ncl, D])
                    )
                    dd = cov_pool.tile(
                        [P, Mg * nr * ncl * D], F32, tag=f"dd{par}"
                    )
                    ddv = dd[:, :].rearrange(
                        "p (m i j d) -> p m i j d", m=Mg, i=nr, j=ncl
                    )
                    sub_a(ddv, pi, pj)
                    nc.scalar.square(dd[:, :], dd[:, :])
                    add_a(reg, ddv[:, :, :, :, 0], ddv[:, :, :, :, 1])
                else:
                    # dx into A (in-place square), dy in an E-sized tmp;
                    # rows split between GPSIMD (top COV_GP_ROWS[ri]) and
                    # DVE for fine-grained engine balance
                    dy = cov_pool.tile(
                        [P, Mg * nr * ncl], F32, tag=f"dd{par}"
                    )
                    dyv = dy[:, :].rearrange(
                        "p (m i j) -> p m i j", m=Mg, i=nr
                    )
                    csplit = r0 + COV_GP_ROWS[ri]
                    for ceng, pa, pb in (("g", r0, csplit), ("v", csplit, r1)):
                        pn = pb - pa
                        if pn <= 0:
                            continue
                        regp = A4[:, :, pa:pb, r0:]
                        dyp = dyv[:, :, pa - r0 : pb - r0]
                        xi = (
                            posv[:, :, pa:pb, 0]
                            .unsqueeze(3)
                            .broadcast_to([P, Mg, pn, ncl])
                        )
                        xj = (
                            posv[:, :, r0:, 0]
                            .unsqueeze(2)
                            .broadcast_to([P, Mg, pn, ncl])
                        )
                        yi = (
                            posv[:, :, pa:pb, 1]
                            .unsqueeze(3)
                            .broadcast_to([P, Mg, pn, ncl])
                        )
                        yj = (
                            posv[:, :, r0:, 1]
                            .unsqueeze(2)
                            .broadcast_to([P, Mg, pn, ncl])
                        )
                        if ceng == "g":
                            _gp_sub(nc, regp, xi, xj)
                            nc.scalar.square(regp, regp)
                            _gp_sub(nc, dyp, yi, yj)
                            nc.scalar.square(dyp, dyp)
                            _gp_add(nc, regp, regp, dyp)
                        else:
                            nc.vector.tensor_sub(regp, xi, xj)
                            nc.scalar.square(regp, regp)
                            nc.vector.tensor_sub(dyp, yi, yj)
                            nc.scalar.square(dyp, dyp)
                            nc.vector.tensor_add(regp, regp, dyp)
                dseg = Av[:, :, r0 * (N + 1) : (r1 - 1) * (N + 1) + 1 : N + 1]
                if COV_PHASED is True or (COV_PHASED == 2 and ri > 0):
                    cov_tail.append((reg, dseg))
                else:
                    nc.scalar.sqrt(reg, reg)
                    nc.scalar.activation(reg, reg, AF.Exp, scale=-PHI)
                    # nugget on this rect's diagonal segment only: pivot k's
                    # reciprocal then depends on rect band(k) alone, letting
                    # early pivots overlap the remaining rects' cov chains
                    if TAU_ENG == "g":
                        nc.gpsimd.tensor_scalar_add(dseg, dseg, TAU)
                    elif TAU_ENG == "a":
                        nc.scalar.add(dseg, dseg, TAU)
                    else:
                        nc.vector.tensor_scalar_add(dseg, dseg, TAU)
                yield
            if cov_tail:
                for reg, _ in cov_tail:
                    nc.scalar.sqrt(reg, reg)
                for reg, _ in cov_tail:
                    nc.scalar.activation(reg, reg, AF.Exp, scale=-PHI)
                for _, dseg in cov_tail:
                    if TAU_ENG == "g":
                        nc.gpsimd.tensor_scalar_add(dseg, dseg, TAU)
                    elif TAU_ENG == "a":
                        nc.scalar.add(dseg, dseg, TAU)
                    else:
                        nc.vector.tensor_scalar_add(dseg, dseg, TAU)
                yield


            # ---- sweep all 20 pivots (gather-free) ----
            # The raw pivot column/row is read straight out of A as
            # broadcast operands of the rank-1 muls (column k for rows
            # above the pivot, row k for rows below); cr (= c * 1/pivot)
            # is computed from A the same way. All muls are emitted
            # before any sub so the in-place subs (which corrupt the
            # pivot row/col: cr[k] = 1) never race the raw reads.
            # The reciprocal for pivot k+1 is issued right after the sub
            # that finalizes A[k+1,k+1], hiding it under pivot k's tail.
            rK = small_pool.tile([P, Mg], F32, tag=f"r{par}")
            nc.vector.reciprocal(rK[:, :], A4[:, :, 0, 0])
            for k in range(N):
                crK = small_pool.tile([P, Mg * N], F32, tag=f"cr{par}")
                cr3 = crK[:, :].rearrange("p (m i) -> p m i", m=Mg)
                rb = rK[:, :].unsqueeze(2).broadcast_to([P, Mg, N])

                def crmul(which, o, a, b):
                    # "s": col part on GPSIMD; "s2": col part alternates
                    # engines by pivot parity; "r": row part on GPSIMD,
                    # col part on DVE; "r2": row part alternates
                    if which == 0 and (
                        CR_ENG == "s" or (CR_ENG == "s2" and k % 2 == 0)
                    ):
                        _gp_mul(nc, o, a, b)
                    elif which == 1 and (
                        CR_ENG == "r" or (CR_ENG == "r2" and k % 2 == 0)
                    ):
                        _gp_mul(nc, o, a, b)
                    elif CR_ENG == "g":
                        _gp_mul(nc, o, a, b)
                    else:
                        nc.vector.tensor_mul(o, a, b)

                if k:
                    crmul(0, cr3[:, :, :k], A4[:, :, :k, k], rb[:, :, :k])
                crmul(1, cr3[:, :, k:], A4[:, :, k, k:], rb[:, :, k:])

                def c_raw(a, b):
                    """Broadcast AP of raw c[a:b] read from A's storage."""
                    if b <= k + 1:  # rows at or above the pivot: column k
                        return A4[:, :, a:b, k].unsqueeze(3)
                    return A4[:, :, k, a:b].unsqueeze(3)  # below: row k

                last = k == N - 1
                subs = []
                gp_rows = GP_ROWS_EVEN if k % 2 == 0 else GP_ROWS
                if FINE_BANDS is not None:
                    band_iter = []
                    for ri, (r0, r1, eng) in enumerate(FINE_BANDS):
                        if eng == "a":  # alternate by pivot+stream parity
                            eng = "v" if (k + s) % 2 else "g"
                        elif eng == "A":
                            eng = "g" if (k + s) % 2 else "v"
                        if (
                            OWNS_V
                            and eng == "g"
                            and r0 <= k + 1 < r1
                            and k < N - 1
                        ):
                            # dedicated DVE piece for the next-pivot row
                            if k + 2 < r1:
                                band_iter.append(
                                    (ri, r0, r1, "g", k + 2, r1)
                                )
                            if r0 < k + 1:
                                band_iter.append(
                                    (ri, r0, r1, "g", r0, k + 1)
                                )
                            band_iter.append(
                                (100 + ri, r0, r1, "v", k + 1, k + 2)
                            )
                        else:
                            band_iter.append((ri, r0, r1, eng, r0, r1))
                else:
                    band_iter = []
                    for ri, (r0, r1) in enumerate(RECTS):
                        gsplit = r0 + gp_rows[ri]
                        band_iter.append((ri, r0, r1, "g", r0, gsplit))
                        band_iter.append((ri, r0, r1, "v", gsplit, r1))
                for ri, r0, r1, eng, a, b in band_iter:
                    ncl = N - r0
                    if True:
                        if b <= a:
                            continue
                        # rows to update: [a,b) minus the pivot row k
                        # (its results are discarded: the pivot row/col
                        # copies rewrite it below). Rows above the pivot
                        # read c from column k, rows below from row k.
                        if a <= k < b:
                            pieces = [(a, k), (k + 1, b)]
                        else:
                            pieces = [(a, b)]
                        pieces = [(pa, pb) for pa, pb in pieces if pb > pa]
                        if not pieces:
                            continue
                        tot = sum(pb - pa for pa, pb in pieces)
                        pool = grect_pool if eng == "g" else rect_pool
                        tmp = pool.tile(
                            [P, Mg * tot * ncl], F32, tag=f"t{eng}{par}r{ri}"
                        )
                        tv = tmp[:, :].rearrange(
                            "p (m i j) -> p m i j", m=Mg, i=tot
                        )
                        mul = (
                            (lambda o, x, y: _gp_mul(nc, o, x, y))
                            if eng == "g"
                            else nc.vector.tensor_mul
                        )
                        ofs = 0
                        for (pa, pb) in pieces:
                            pn = pb - pa
                            crb = (
                                cr3[:, :, r0:]
                                .unsqueeze(2)
                                .broadcast_to([P, Mg, pn, ncl])
                            )
                            mul(
                                tv[:, :, ofs : ofs + pn],
                                c_raw(pa, pb).broadcast_to([P, Mg, pn, ncl]),
                                crb,
                            )
                            # does this finalize next pivot's diagonal?
                            owns_next = pa <= k + 1 < pb
                            subs.append(
                                (
                                    eng,
                                    A4[:, :, pa:pb, r0:],
                                    tv[:, :, ofs : ofs + pn],
                                    owns_next,
                                )
                            )
                            ofs += pn
                subs.sort(key=lambda t: not t[3])  # next-diag owner first
                for si, (eng, reg, tv, owns_next) in enumerate(subs):
                    if eng == "g":
                        if last:
                            # fold the final negation into the last
                            # pivot: reg <- tv - reg = -(reg - tv)
                            _gp_sub(nc, reg, tv, reg)
                        else:
                            _gp_sub(nc, reg, reg, tv)
                    else:
                        if last:
                            nc.vector.tensor_sub(reg, tv, reg)
                        else:
                            nc.vector.tensor_sub(reg, reg, tv)
                    if owns_next and not last:
                        rK_next = small_pool.tile(
                            [P, Mg], F32, tag=f"r{par}"
                        )
                        nc.vector.reciprocal(
                            rK_next[:, :], A4[:, :, k + 1, k + 1]
                        )
                # pivot row/col (upper parts) <- cr; diag <- -r
                # (the last pivot writes negated values: the whole rect
                # cover holds -result after its reverse subtract)
                if k < N - 1:
                    pcopy = {
                        "v": nc.vector.tensor_copy,
                        "a": nc.scalar.copy,
                        "g": nc.gpsimd.tensor_copy,
                    }[PIVOT_COPY_ENG]
                    if k:
                        pcopy(A4[:, :, :k, k], cr3[:, :, :k])
                    pcopy(A4[:, :, k, k + 1 :], cr3[:, :, k + 1 :])
                    if DIAG_ENG == "a":
                        nc.scalar.mul(A4[:, :, k, k], rK[:, :], -1.0)
                    elif DIAG_ENG == "g":
                        nc.gpsimd.tensor_scalar_mul(
                            A4[:, :, k, k], rK[:, :], -1.0
                        )
                    else:
                        nc.vector.tensor_scalar_mul(
                            A4[:, :, k, k], rK[:, :], -1.0
                        )
                else:
                    if PIVOT_COPY_ENG == "a":
                        nc.scalar.mul(A4[:, :, :k, k], cr3[:, :, :k], -1.0)
                    else:
                        nc.vector.tensor_scalar_mul(
                            A4[:, :, :k, k], cr3[:, :, :k], -1.0
                        )
                    nc.vector.tensor_copy(A4[:, :, k, k], rK[:, :])
                if k < N - 1:
                    rK = rK_next
                yield

            # ---- finalize: mirror upper -> lower (values already negated),
            # in m-halves so the first half's store overlaps the second
            # half's mirror. The last stream has no concurrent work left,
            # so its mirror runs split across the otherwise-idle DVE+GPSIMD.
            tail = s == n_streams - 1
            h = Mg // 2
            for hi, (m0, m1) in enumerate(((0, h), (h, Mg))):
                for i in range(N - 1):
                    if tail:
                        mcopy = (
                            nc.vector.tensor_copy
                            if i % 2
                            else nc.gpsimd.tensor_copy
                        )
                    elif MIRROR_ENG == "a":
                        mcopy = nc.scalar.copy
                    else:
                        mcopy = nc.vector.tensor_copy
                    mcopy(
                        A4[:, m0:m1, i + 1 :, i], A4[:, m0:m1, i, i + 1 :]
                    )
                # the last stream's second half goes out via the ACT
                # HWDGE queue so both halves' transfers overlap (ACT is
                # idle during the pipeline drain; mid-run streams stay on
                # the SP queue where the issue cost is off-engine)
                dma_eng = nc.scalar if (tail and hi == 1) else nc.sync
                dma_eng.dma_start(
                    out_r[:, offs[s] + m0 : offs[s] + m1, :],
                    A[:, m0 * N * N : m1 * N * N],
                )
                yield

        pending = list(range(n_streams))
        active = [stream_gen(pending.pop(0))]
        if COV_PAR_START and pending:
            # run both initial streams' cov builds in parallel, then
            # prime stream 0's sweep so pivots stay phase-offset
            active.append(stream_gen(pending.pop(0)))
            for _ in range(len(RECTS)):
                for gen in active:
                    next(gen)
            for _ in range(STAGGER):
                next(active[0])
        else:
            # prime the first stream so concurrent streams stay offset
            for _ in range(STAGGER):
                next(active[0])
        while pending or active:
            while len(active) < ACTIVE and pending:
                active.append(stream_gen(pending.pop(0)))
            for gen in list(active):
                try:
                    next(gen)
                except StopIteration:
                    active.remove(gen)


_CACHE = {}


def build_nc(b_core=B_CORE, n_streams=None, num_devices=N_CORES):
    if n_streams is None:
        n_streams = N_STREAMS
    key = (b_core, n_streams, num_devices)
    if key in _CACHE:
        return _CACHE[key]
    nc = bacc.Bacc(
        "TRN2", target_bir_lowering=False, debug=False, num_devices=num_devices
    )
    pos_d = nc.dram_tensor("pos", [b_core, N * D], F32, kind="ExternalInput")
    out_d = nc.dram_tensor("out", [b_core, N * N], F32, kind="ExternalOutput")
    with tile.TileContext(nc) as tc:
        emit_kernel(tc, pos_d.ap(), out_d.ap(), b_core, n_streams)
    nc.compile()
    _CACHE[key] = nc
    return nc


def run(pos_full, b_core=B_CORE, n_streams=None, n_cores=N_CORES, **kw):
    """pos_full: [n_cores*b_core, 20, 2] f32 -> [n_cores*b_core, 20, 20] f32."""
    nc = build_nc(b_core, n_streams, n_cores)
    flat = np.ascontiguousarray(
        np.asarray(pos_full, dtype=np.float32).reshape(-1, N * D)
    )
    in_maps = [
        {"pos": flat[i * b_core : (i + 1) * b_core]} for i in range(n_cores)
    ]
    res = run_bass_kernel_spmd(nc, in_maps, core_ids=list(range(n_cores)), **kw)
    out = np.concatenate([r["out"] for r in res.results], axis=0)
    return out.reshape(-1, N, N), res


def kernel(neighbor_positions, edge_list=None):
    out, _ = run(neighbor_positions)
    return out

